# revision 1
# baseline (speedup 1.0000x reference)
"""Multi-head attention (N=2, L=2048, D=1024, H=16) on 8 NeuronCores.

Sharding: core c -> (batch n = c // 4, head group g = c % 4, 4 heads each).
Each core computes Q/K/V projections for its 4 heads, attention, and its
slice of the output projection. Host sums the 4 partial output projections
per batch and adds bo.

Design: each core computes Q/K/V projections for its 4 heads (Q packs a
head pair into one M=128 matmul per c-chunk), flash-style attention
(S^T = K @ Q^T per k-tile with the unused head-half of Q zeroed, exp on
ScalarE with 1/sqrt(D) folded into the activation scale, P^T @ V via
TensorE with a ones-column appended to V so the softmax denominator
accumulates in PSUM row 64 for free), 1/denom via Ln then Exp(-x) on
ScalarE (off the PE/DVE critical path; both functions share the
natural_log_exp table set), a PE broadcast of the reciprocal, and the
per-head zero-padded output projection. Output DMA is fp16; the host
sums the 4 partial output projections per batch in fp32 and adds bo.

All matmul operands are fp16 (full-speed PE, fp32 PSUM accumulate).

Notes from profiling (for future work): the kernel is TensorE-bound
(~211us busy vs ~135us ScalarE exp). tile_position row/col tiling does
NOT overlap matmuls here: LDWEIGHTS only pulls ahead when PE row groups
are disjoint, and streams only overlap when column strips are disjoint
AND the instructions issue back-to-back — K=128 matmuls (PV, denom)
serialize on the weight load no matter the col tiling, and the tile
scheduler's readiness-driven ordering splits concurrency groups anyway.
Single-partition DVE ops at a partition offset (e.g. [1,512] at row 64)
silently no-op on this stack — use ScalarE for single-row math.
Startup is ~24us of input-DMA wait before the first matmul. A qt-major
block layout of x^T (one [128, DC, 512] tile + DMA per 512-token block,
8KB contiguous lines, projections unblock after ~1MB) measured
252-257us — but produced 100%-NaN output on core 0 only, reproducibly
(3 runs, two tile-layout variants). That is NOT a layout/numerics bug:
cores 1-3 consume byte-identical x arrays (shared batch 0) and pass;
it is specific to the NTFF-profiled core. To adopt the ~8us win, first
bisect trace=False behavior vs the profiled re-execution path in
run_bass_kernel_spmd. Also: keep >=4KB per-partition DMA lines —
slicing xk/xv into [128,512] pieces (1KB lines) cost +15us.
"""
import os
import sys
import types

import numpy as np

N_BATCH = 2
L = 2048
D = 1024
H = 16
HD = 64
CORES = 8
GH = 4            # heads per core
DG = GH * HD      # 256 = projected dims per core
QB = 512          # q block
KT = L // 128     # 16 k tiles
QT = L // QB      # 4 q blocks
DC = D // 128     # 8 din chunks
SCALE = 1.0 / 32.0  # 1/sqrt(D)


def _install_ntff_hook():
    """The image's antenv stub lacks axon_hooks; shim it so trace=True works."""
    if "antenv.axon_hooks" in sys.modules:
        return
    mod = types.ModuleType("antenv.axon_hooks")
    mod._hook = None
    mod.set_axon_ntff_profile_hook = lambda h: setattr(mod, "_hook", h)
    mod.get_axon_ntff_profile_hook = lambda: mod._hook
    sys.modules["antenv.axon_hooks"] = mod
    try:
        from trn_agent_boot.trn_boot import _ntff_profile_via_ctypes
        mod._hook = _ntff_profile_via_ctypes("/opt/axon/libaxon_pjrt.so")
    except Exception:
        mod._hook = None


_install_ntff_hook()

import concourse.bacc as bacc
import concourse.mybir as mybir
import concourse.tile as tile
from concourse.bass_utils import run_bass_kernel_spmd

F32 = mybir.dt.float32
F16 = mybir.dt.float16
F32R = mybir.dt.float32r
AF = mybir.ActivationFunctionType
MULT = mybir.AluOpType.mult

_CACHE = {}


_TABLES_PATCHED = False


def _patch_act_tables():
    """Prefer natural_log_exp_and_others so Exp and Ln share one table set."""
    global _TABLES_PATCHED
    if _TABLES_PATCHED:
        return
    import concourse.bacc as _bacc
    import concourse.hw_specs as _hw
    orig_fn = _hw.get_activation_tables

    def patched(arch):
        import concourse.mybir as _mybir
        tabs = dict(orig_fn(arch))
        pref = "natural_log_exp_and_others"
        if pref not in tabs:
            return tabs
        drop = {_mybir.ActivationFunctionType.Exp,
                _mybir.ActivationFunctionType.Ln}
        return {k: (v if k == pref else (set(v) - drop))
                for k, v in tabs.items()}

    _bacc.get_activation_tables = patched
    _TABLES_PATCHED = True


def _build(use_bias, use_mask):
    key = (use_bias, use_mask)
    if key in _CACHE:
        return _CACHE[key]
    _patch_act_tables()

    nc = bacc.Bacc("TRN2", debug=False, num_devices=CORES)

    xqT = nc.dram_tensor("xqT", [D, L], F16, kind="ExternalInput").ap()
    xkT = nc.dram_tensor("xkT", [D, L], F16, kind="ExternalInput").ap()
    xvT = nc.dram_tensor("xvT", [D, L], F16, kind="ExternalInput").ap()
    aq = nc.dram_tensor("aq", [128, DC * DG], F16, kind="ExternalInput").ap()
    ak = nc.dram_tensor("ak", [128, DC * DG], F16, kind="ExternalInput").ap()
    av = nc.dram_tensor("av", [128, DC * DG], F16, kind="ExternalInput").ap()
    bo = nc.dram_tensor("bo", [128, GH * D], F16, kind="ExternalInput").ap()
    bq = nc.dram_tensor("bq", [1, DG], F16, kind="ExternalInput").ap()
    bk = nc.dram_tensor("bk", [1, DG], F16, kind="ExternalInput").ap()
    bv = nc.dram_tensor("bv", [1, DG], F16, kind="ExternalInput").ap()
    maskf = nc.dram_tensor("maskf", [128, KT], F32, kind="ExternalInput").ap()
    onesd = nc.dram_tensor("onesd", [128, 512], F16, kind="ExternalInput").ap()
    outp = nc.dram_tensor("outp", [L, D], F16, kind="ExternalOutput").ap()

    with tile.TileContext(nc) as tc:
        _emit(nc, tc, dict(xqT=xqT, xkT=xkT, xvT=xvT, aq=aq, ak=ak, av=av,
                           bo=bo, bq=bq, bk=bk, bv=bv, maskf=maskf,
                           onesd=onesd, outp=outp),
              use_bias, use_mask)
    nc.compile()
    _CACHE[key] = nc
    return nc


def _emit(nc, tc, t, use_bias, use_mask):
    from contextlib import ExitStack
    ctx = ExitStack()
    with ctx:
        sb_w = ctx.enter_context(tc.tile_pool(name="sb_w", bufs=1))
        sb_qkv = ctx.enter_context(tc.tile_pool(name="sb_qkv", bufs=1))
        sb_pt = ctx.enter_context(tc.tile_pool(name="sb_pt", bufs=6))
        sb_n = ctx.enter_context(tc.tile_pool(name="sb_n", bufs=8))
        sb_out = ctx.enter_context(tc.tile_pool(name="sb_out", bufs=3))
        ps = ctx.enter_context(tc.tile_pool(name="ps", bufs=8, space="PSUM"))

        # ---- resident tiles ----
        ak_t = sb_w.tile([128, DC, DG], F16, tag="ak")
        aq_t = sb_w.tile([128, DC, DG], F16, tag="aq")
        av_t = sb_w.tile([128, DC, DG], F16, tag="av")
        bo_t = sb_w.tile([128, GH, D], F16, tag="bo")
        ones_t = sb_w.tile([128, 512], F16, tag="ones")
        xk_res = sb_w.tile([128, DC, L], F16, tag="xk")
        xq_res = sb_w.tile([128, DC, L], F16, tag="xq")
        xv_res = sb_w.tile([128, DC, L], F16, tag="xv")
        KT_sb = [sb_qkv.tile([128, L], F16, tag=f"kt{m}", name=f"KTm{m}")
                 for m in range(2)]
        QT_z = [sb_qkv.tile([128, L], F16, tag=f"qz{h}", name=f"QTz{h}")
                for h in range(GH)]
        V1 = sb_qkv.tile([128, KT, GH, HD + 1], F16, tag="v1")
        oN_sb = [sb_qkv.tile([128, 512], F16, tag=f"oN{h}", name=f"oN{h}")
                 for h in range(GH)]

        # ---- input DMAs: one priority-ordered queue (sync) ----
        # (weights are host-preswizzled to [128, free] partition-contiguous)
        nc.sync.dma_start(out=aq_t, in_=t["aq"].rearrange("p (c d) -> p c d", c=DC))
        for c in range(DC):  # qb0 slices of xq first: unblocks qproj(0)
            nc.sync.dma_start(
                out=xq_res[:, c, 0:512], in_=t["xqT"][c * 128:(c + 1) * 128, 0:512])
        nc.sync.dma_start(out=ak_t, in_=t["ak"].rearrange("p (c d) -> p c d", c=DC))
        for c in range(DC):
            nc.sync.dma_start(out=xk_res[:, c, :],
                              in_=t["xkT"][c * 128:(c + 1) * 128, :])
        nc.sync.dma_start(out=av_t, in_=t["av"].rearrange("p (c d) -> p c d", c=DC))
        for c in range(DC):
            nc.sync.dma_start(out=xv_res[:, c, :],
                              in_=t["xvT"][c * 128:(c + 1) * 128, :])
        nc.sync.dma_start(out=ones_t, in_=t["onesd"])
        if use_mask:
            mask_t = sb_w.tile([128, KT], F32, tag="mask")
            nc.sync.dma_start(out=mask_t, in_=t["maskf"])
        for qt in range(1, QT):
            for c in range(DC):
                nc.sync.dma_start(
                    out=xq_res[:, c, qt * 512:(qt + 1) * 512],
                    in_=t["xqT"][c * 128:(c + 1) * 128, qt * 512:(qt + 1) * 512])
        nc.sync.dma_start(out=bo_t, in_=t["bo"].rearrange("p (a d) -> p a d", a=GH))
        bq_t = bk_t = bv_t = None
        if use_bias:
            bq_t = sb_w.tile([1, DG], F16, tag="bq")
            bk_t = sb_w.tile([1, DG], F16, tag="bk")
            bv_t = sb_w.tile([1, DG], F16, tag="bv")
            nc.sync.dma_start(out=bq_t, in_=t["bq"])
            nc.sync.dma_start(out=bk_t, in_=t["bk"])
            nc.sync.dma_start(out=bv_t, in_=t["bv"])

        # ACT table warmup (exp only)
        warm = sb_w.tile([1, 32], F32, tag="warm")
        nc.vector.memset(warm, 1.0)
        warm2 = sb_w.tile([1, 32], F32, tag="warm2")
        nc.scalar.activation(out=warm2, in_=warm, func=AF.Ln)
        nc.scalar.activation(out=warm2, in_=warm, func=AF.Exp)

        for h in range(GH):
            nc.vector.memset(oN_sb[h][64:128, :], 0.0)
        for h in range(GH):
            z0 = 0 if h % 2 else 64
            nc.vector.memset(QT_z[h][z0:z0 + 64, :], 0.0)

        # V1 ones column (column HD of every (kt, h) slot)
        if use_mask:
            ones4 = sb_w.tile([128, GH], F32, tag="ones4")
            nc.vector.memset(ones4, 1.0)
            for kt in range(KT):
                nc.vector.tensor_scalar_mul(
                    V1[:, kt, :, HD:HD + 1],
                    ones4.rearrange("p h -> p h 1"), mask_t[:, kt:kt + 1])
        else:
            nc.sync.dma_start(
                out=V1[:, :, :, HD:HD + 1],
                in_=t["onesd"][:, 0:KT * GH].rearrange(
                    "p (a b c) -> p a b c", a=KT, c=1))

        # ---- emit helpers ----
        def emit_kproj(qt):
            # per m half: accumulate over c chunks -> KT_sb[m]
            psm = [ps.tile([128, 512], F32, tag="o", bufs=4,
                           name=f"psk_{qt}_{_}") for _ in range(2)]
            for m in range(2):
                for c in range(DC):
                    xsl = xk_res[:, c, qt * 512:(qt + 1) * 512]
                    nc.tensor.matmul(
                        psm[m][:, 0:512], ak_t[:, c, m * 128:(m + 1) * 128], xsl,
                        start=(c == 0), stop=(c == DC - 1 and not use_bias))
                if use_bias:
                    nc.tensor.matmul(
                        psm[m][:, 0:512], bk_t[:, m * 128:(m + 1) * 128],
                        ones_t[0:1, :], start=False, stop=True)
                nc.vector.tensor_copy(
                    KT_sb[m][:, qt * 512:(qt + 1) * 512], psm[m][:, 0:512])

        def emit_qproj(qt, p):
            # packed head pair p: one M=128 matmul per c chunk
            psq = ps.tile([128, 512], F32, tag="o", bufs=4, name=f"psq_{qt}_{p}")
            for c in range(DC):
                xsl = xq_res[:, c, qt * 512:(qt + 1) * 512]
                nc.tensor.matmul(
                    psq[:, 0:512], aq_t[:, c, p * 128:(p + 1) * 128], xsl,
                    start=(c == 0), stop=(c == DC - 1 and not use_bias))
            if use_bias:
                nc.tensor.matmul(
                    psq[:, 0:512], bq_t[:, p * 128:(p + 1) * 128],
                    ones_t[0:1, :], start=False, stop=True)
            for hh in range(2):
                h = p * 2 + hh
                r0 = 64 * hh
                nc.vector.tensor_copy(
                    QT_z[h][r0:r0 + 64, qt * 512:(qt + 1) * 512],
                    psq[r0:r0 + 64, 0:512])

        def emit_vproj(ktg, jp):
            js = (jp * 2, jp * 2 + 1)
            psv = {j: ps.tile([128, 512], F32, tag="o", bufs=4,
                              name=f"psv_{ktg}_{j}") for j in js}
            for j in js:
                for c in range(DC):
                    xsl = xv_res[:, c, ktg * 512:(ktg + 1) * 512]
                    nc.tensor.matmul(
                        psv[j][:, 0:DG], xsl[:, j * 128:(j + 1) * 128],
                        av_t[:, c, :],
                        start=(c == 0), stop=(c == DC - 1 and not use_bias))
                if use_bias:
                    nc.tensor.matmul(
                        psv[j][:, 0:DG], ones_t[0:1, 0:128], bv_t,
                        start=False, stop=True)
                kt = ktg * 4 + j
                srcv = psv[j][:, 0:DG].rearrange("p (h d) -> p h d", h=GH)
                if use_mask:
                    nc.vector.tensor_scalar_mul(
                        V1[:, kt, :, 0:HD], srcv, mask_t[:, kt:kt + 1])
                else:
                    nc.vector.tensor_copy(V1[:, kt, :, 0:HD], srcv)

        def emit_attn_sk(qb, sk, pso):
            qs0 = qb * QB
            pss = {}
            pts = {}
            # S^T: K=128 with the unused head-half of Q zeroed, so head
            # pairs share the K-pair lhsT tile (baseline scheme)
            for hp in range(2):
                for dk in range(2):
                    kt = sk * 2 + dk
                    for hh in range(2):
                        h = hp * 2 + hh
                        if h not in pss:
                            pss[h] = ps.tile([128, 1024], F32, tag="s", bufs=2,
                                             name=f"pss_{qb}_{sk}_{h}")
                        nc.tensor.matmul(
                            pss[h][:, dk * 512:(dk + 1) * 512],
                            KT_sb[hp][:, kt * 128:(kt + 1) * 128],
                            QT_z[h][:, qs0:qs0 + QB],
                            start=True, stop=True)
                for hh in range(2):
                    h = hp * 2 + hh
                    pt = sb_pt.tile([128, 1024], F16, tag="pt",
                                    name=f"pt_{qb}_{sk}_{h}")
                    nc.scalar.activation(out=pt, in_=pss[h], func=AF.Exp,
                                         scale=SCALE)
                    pts[h] = pt
            # PV: M=65 per head — 64 V columns + a ones column that
            # accumulates the softmax denominator for free (row 64)
            for dk in range(2):
                kt = sk * 2 + dk
                for h in range(GH):
                    nc.tensor.matmul(
                        pso[h][0:HD + 1, :], V1[:, kt, h, :],
                        pts[h][:, dk * 512:(dk + 1) * 512],
                        start=(kt == 0), stop=(kt == KT - 1))

        oT_all = {}

        def emit_oT(qb):
            # copy PV accumulators (V rows + denom row) to SBUF, frees psum;
            # reciprocal of the denominator row on DVE (in place, row 64)
            oTs = []
            for h in range(GH):
                oT = sb_n.tile([65, 512], F32, tag="oT", name=f"oT_{qb}_{h}")
                nc.vector.tensor_copy(oT, pso_all[qb][h][0:65, :])
                oTs.append(oT)
            oT_all[qb] = oTs

        def emit_tail(qb):
            qs0 = qb * QB
            oTs = oT_all[qb]
            for h in range(GH):
                oT = oTs[h]
                # 1/denom via Ln then Exp(-x) on ScalarE (off the PE/DVE
                # critical path), then broadcast down 64 partitions via PE
                lnr = sb_n.tile([65, 512], F32, tag="lnr", bufs=4,
                                name=f"lnr_{qb}_{h}")
                nc.scalar.activation(out=lnr[64:65, :], in_=oT[64:65, :],
                                     func=AF.Ln)
                rr = sb_n.tile([65, 512], F16, tag="rr", bufs=4,
                               name=f"rr_{qb}_{h}")
                nc.scalar.activation(out=rr[64:65, :], in_=lnr[64:65, :],
                                     func=AF.Exp, scale=-1.0)
                bc = ps.tile([128, 512], F32, tag="o", bufs=4,
                             name=f"bc_{qb}_{h}")
                nc.tensor.matmul(
                    bc[0:64, :], ones_t[64:65, 0:64], rr[64:65, :],
                    start=True, stop=True, tile_position=(64, 0))
                nc.vector.tensor_tensor(oN_sb[h][0:64, :], oT[0:64, :],
                                        bc[0:64, :], op=MULT)
            for mq in range(4):
                ot = sb_out.tile([128, D], F16, tag="ot", name=f"ot_{qb}_{mq}")
                for nb in range(2):
                    psout = ps.tile([128, 512], F32, tag="o", bufs=4,
                                    name=f"psout_{qb}_{mq}_{nb}")
                    for h in range(GH):
                        nc.tensor.matmul(
                            psout[:, 0:512],
                            oN_sb[h][:, mq * 128:(mq + 1) * 128],
                            bo_t[:, h, nb * 512:(nb + 1) * 512],
                            start=(h == 0), stop=(h == GH - 1))
                    nc.vector.tensor_copy(ot[:, nb * 512:(nb + 1) * 512],
                                          psout[:, 0:512])
                q0 = qs0 + mq * 128
                nc.gpsimd.dma_start(out=t["outp"][q0:q0 + 128, :], in_=ot)

        # ---- schedule ----
        pso_all = {}
        emit_qproj(0, 0)
        emit_qproj(0, 1)
        for g in range(4):
            emit_kproj(g)
        for g in range(4):
            emit_vproj(g, 0)
            emit_vproj(g, 1)
        pso_all[0] = [ps.tile([128, 512], F32, tag="o", bufs=4,
                              name=f"pso_0_{_}") for _ in range(GH)]
        for sk in range(KT // 2):
            emit_attn_sk(0, sk, pso_all[0])
        emit_oT(0)
        for qb in range(1, QT):
            emit_qproj(qb, 0)
            emit_qproj(qb, 1)
            emit_tail(qb - 1)
            pso_all[qb] = [ps.tile([128, 512], F32, tag="o", bufs=4,
                                   name=f"pso_{qb}_{_}") for _ in range(GH)]
            for sk in range(KT // 2):
                emit_attn_sk(qb, sk, pso_all[qb])
            emit_oT(qb)
        emit_tail(QT - 1)


def _swizzle_a(aT):
    """[D, DG] -> [128, DC*DG]: partition p holds chunks c at (c, :)."""
    return np.ascontiguousarray(
        aT.reshape(DC, 128, DG).transpose(1, 0, 2).reshape(128, DC * DG))


def _pad_bo(boT):
    """[256, D] -> [128, GH*D]: head h cols at h*D, rows 64:128 zero."""
    out = np.zeros((128, GH, D), dtype=np.float16)
    out[0:64, :, :] = boT.reshape(GH, 64, D).transpose(1, 0, 2)
    return np.ascontiguousarray(out.reshape(128, GH * D))


_ONES = np.ones((128, 512), dtype=np.float16)


def _prep_inputs(values, key, query, mask, Wv, Wk, Wq, Wo, bv, bk, bq):
    """Build the 8 per-core input maps (host-side shard + layout)."""
    xT = {}
    for n in range(N_BATCH):
        xT[("q", n)] = np.ascontiguousarray(query[n].T.astype(np.float16))
        xT[("k", n)] = np.ascontiguousarray(key[n].T.astype(np.float16))
        xT[("v", n)] = np.ascontiguousarray(values[n].T.astype(np.float16))
    in_maps = []
    for c in range(CORES):
        n, g = divmod(c, CORES // N_BATCH)
        rows = slice(g * DG, (g + 1) * DG)
        mrow = np.ascontiguousarray(
            mask[n, 0, 0, :].astype(np.float32).reshape(KT, 128).T)
        boT = _pad_bo(Wo[:, rows].T.astype(np.float16))
        in_maps.append({
            "xqT": xT[("q", n)],
            "xkT": xT[("k", n)],
            "xvT": xT[("v", n)],
            "aq": _swizzle_a(Wq[rows, :].T.astype(np.float16)),
            "ak": _swizzle_a(Wk[rows, :].T.astype(np.float16)),
            "av": _swizzle_a(Wv[rows, :].T.astype(np.float16)),
            "bo": boT,
            "bq": np.ascontiguousarray(bq[None, rows].astype(np.float16)),
            "bk": np.ascontiguousarray(bk[None, rows].astype(np.float16)),
            "bv": np.ascontiguousarray(bv[None, rows].astype(np.float16)),
            "maskf": mrow,
            "onesd": _ONES,
        })
    return in_maps


LAST_EXEC_NS = None
LAST_RES = None


def kernel(values, key, query, mask, Wv, bv, Wk, bk, Wq, bq, Wo, bo,
           trace=False):
    global LAST_EXEC_NS, LAST_RES
    values = np.asarray(values, dtype=np.float32)
    key = np.asarray(key, dtype=np.float32)
    query = np.asarray(query, dtype=np.float32)
    mask = np.asarray(mask)
    Wq, Wk, Wv, Wo = (np.asarray(Wq, np.float32), np.asarray(Wk, np.float32),
                      np.asarray(Wv, np.float32), np.asarray(Wo, np.float32))
    bq, bk, bv, bo = (np.asarray(bq, np.float32), np.asarray(bk, np.float32),
                      np.asarray(bv, np.float32), np.asarray(bo, np.float32))

    use_bias = bool(np.any(bq) or np.any(bk) or np.any(bv))
    use_mask = not bool(np.all(np.asarray(mask) == 1))

    nc = _build(use_bias, use_mask)
    in_maps = _prep_inputs(values, key, query, mask, Wv, Wk, Wq, Wo,
                           bv, bk, bq)
    res = run_bass_kernel_spmd(nc, in_maps, core_ids=list(range(CORES)),
                               trace=trace)
    LAST_EXEC_NS = res.exec_time_ns
    LAST_RES = res

    out = np.zeros((N_BATCH, L, D), dtype=np.float32)
    for c in range(CORES):
        n = c // (CORES // N_BATCH)
        out[n] += res.results[c]["outp"].astype(np.float32)
    out += bo[None, None, :]
    return out



# revision 6
# speedup vs baseline: 1.0053x; 1.0053x over previous
"""Multi-head attention (N=2, L=2048, D=1024, H=16) on 8 NeuronCores.

Sharding: core c -> (batch n = c // 4, head group g = c % 4, 4 heads each).
Each core computes Q/K/V projections for its 4 heads, attention, and its
slice of the output projection. Host sums the 4 partial output projections
per batch and adds bo.

v2 design (from baseline profiling: PE 210.7us busy, exec 260us, 31.5us
PE gaps + ~25us HAM cold-clock tax + 12.4us tail):
- Host-blocked qt-major x layouts ([QT, 128, DC*512], 8KB DMA lines) so
  inputs arrive in consumption order; kproj(0)/vproj(0)/qproj(0) feed at
  ~13-19us and the attention pipeline ignites at ~22us.
- Single-head rounds: per (qb, hp, sk, hh): S^T = 2 MMs into one
  [128,1024] fp32 psum tile (ring 2), one exp (ScalarE, scale=1/32),
  PV = 2 MMs (M=65: V dims + ones column accumulating the softmax
  denominator). Ring granularity == round granularity so the exp stream
  never stalls on psum (baseline allocated 4 tiles/round vs ring 2).
- Head-pair outer loop (hp): only 2 PV accumulators live -> PSUM fits:
  s(2x2 banks) + acc(2x1) + o(2x1) = 8 banks, leaving an "o" ring for
  interleaved filler matmuls (projections + prev-qb out-proj).
- Out-proj packs head pairs: oN2[hp] [128,512] holds both heads' dims ->
  K=128 fully used, 8 MMs/qb instead of 16. Odd head's normalized rows
  are written at partition offset 64 (SHIFT_MODE selects DVE direct
  offset write vs PE identity-shift matmul).
- Reciprocal on DVE: bc matmul broadcasts the RAW denominator row (K=1
  matmul from partition 64), nc.vector.reciprocal on [64,512], then one
  fused tensor_tensor multiply -> ScalarE runs exps only (no Ln/Exp
  pair, no act-table patching).
- PE warmup matmuls on memset tiles during the DMA lead-in (HAM).
"""
import os
import sys
import types

import numpy as np

N_BATCH = 2
L = 2048
D = 1024
H = 16
HD = 64
CORES = 8
GH = 4            # heads per core
DG = GH * HD      # 256 = projected dims per core
QB = 512          # q block
KT = L // 128     # 16 k tiles
QT = L // QB      # 4 q blocks
DC = D // 128     # 8 din chunks
SCALE = 1.0 / 32.0  # 1/sqrt(D)
SHIFT_MODE = "dve"  # "dve": direct offset write; "pe": identity matmul shift


def _install_ntff_hook():
    """The image's antenv stub lacks axon_hooks; shim it so trace=True works."""
    if "antenv.axon_hooks" in sys.modules:
        return
    mod = types.ModuleType("antenv.axon_hooks")
    mod._hook = None
    mod.set_axon_ntff_profile_hook = lambda h: setattr(mod, "_hook", h)
    mod.get_axon_ntff_profile_hook = lambda: mod._hook
    sys.modules["antenv.axon_hooks"] = mod
    try:
        from trn_agent_boot.trn_boot import _ntff_profile_via_ctypes
        mod._hook = _ntff_profile_via_ctypes("/opt/axon/libaxon_pjrt.so")
    except Exception:
        mod._hook = None


_install_ntff_hook()

import concourse.bacc as bacc
import concourse.mybir as mybir
import concourse.tile as tile
from concourse.bass_utils import run_bass_kernel_spmd

F32 = mybir.dt.float32
F16 = mybir.dt.float16
AF = mybir.ActivationFunctionType
MULT = mybir.AluOpType.mult

_CACHE = {}


def _build(use_bias, use_mask):
    key = (use_bias, use_mask)
    if key in _CACHE:
        return _CACHE[key]

    nc = bacc.Bacc("TRN2", debug=False, num_devices=CORES)

    xqb = nc.dram_tensor("xqb", [QT * 128, DC * 512], F16, kind="ExternalInput").ap()
    xkb = nc.dram_tensor("xkb", [QT * 128, DC * 512], F16, kind="ExternalInput").ap()
    xvb = nc.dram_tensor("xvb", [QT * 128, DC * 512], F16, kind="ExternalInput").ap()
    aq = nc.dram_tensor("aq", [128, DC * DG], F16, kind="ExternalInput").ap()
    ak = nc.dram_tensor("ak", [128, DC * DG], F16, kind="ExternalInput").ap()
    av = nc.dram_tensor("av", [128, DC * DG], F16, kind="ExternalInput").ap()
    bo = nc.dram_tensor("bo", [128, 2 * D], F16, kind="ExternalInput").ap()
    bq = nc.dram_tensor("bq", [1, DG], F16, kind="ExternalInput").ap()
    bk = nc.dram_tensor("bk", [1, DG], F16, kind="ExternalInput").ap()
    bv = nc.dram_tensor("bv", [1, DG], F16, kind="ExternalInput").ap()
    eye = nc.dram_tensor("eye", [64, 64], F16, kind="ExternalInput").ap()
    maskf = nc.dram_tensor("maskf", [128, KT], F32, kind="ExternalInput").ap()
    outp = nc.dram_tensor("outp", [L, D], F16, kind="ExternalOutput").ap()

    with tile.TileContext(nc) as tc:
        _emit(nc, tc, dict(xqb=xqb, xkb=xkb, xvb=xvb, aq=aq, ak=ak, av=av,
                           bo=bo, bq=bq, bk=bk, bv=bv, eye=eye, maskf=maskf,
                           outp=outp),
              use_bias, use_mask)
    nc.compile()
    _CACHE[key] = nc
    return nc


def _emit(nc, tc, t, use_bias, use_mask):
    from contextlib import ExitStack
    ctx = ExitStack()
    with ctx:
        sb_w = ctx.enter_context(tc.tile_pool(name="sb_w", bufs=1))
        sb_qkv = ctx.enter_context(tc.tile_pool(name="sb_qkv", bufs=1))
        sb_pt = ctx.enter_context(tc.tile_pool(name="sb_pt", bufs=4))
        sb_n = ctx.enter_context(tc.tile_pool(name="sb_n", bufs=4))
        sb_out = ctx.enter_context(tc.tile_pool(name="sb_out", bufs=3))
        ps = ctx.enter_context(tc.tile_pool(name="ps", bufs=2, space="PSUM"))

        # ---- resident tiles ----
        aq_t = sb_w.tile([128, DC, DG], F16, tag="aq")
        ak_t = sb_w.tile([128, DC, DG], F16, tag="ak")
        av_t = sb_w.tile([128, DC, DG], F16, tag="av")
        bo_t = sb_w.tile([128, 2, D], F16, tag="bo")
        ones_t = sb_w.tile([128, 512], F16, tag="ones")
        eye_t = sb_w.tile([64, 64], F16, tag="eye")
        xq_res = sb_w.tile([128, QT, DC * 512], F16, tag="xq")
        xk_res = sb_w.tile([128, QT, DC * 512], F16, tag="xk")
        xv_res = sb_w.tile([128, QT, DC * 512], F16, tag="xv")
        KT_sb = [sb_qkv.tile([128, L], F16, tag=f"kt{m}", name=f"KTm{m}")
                 for m in range(2)]
        QT_z = [sb_qkv.tile([128, L], F16, tag=f"qz{h}", name=f"QTz{h}")
                for h in range(GH)]
        V1 = sb_qkv.tile([128, KT, GH, HD + 1], F16, tag="v1")
        # oN2[qb%2][hp]: packed normalized heads for the out-projection
        oN2 = [[sb_qkv.tile([128, 512], F16, tag=f"oN{b}{hp}",
                            name=f"oN{b}{hp}") for hp in range(2)]
               for b in range(2)]

        # ---- warmup tiles (no DMA deps; HAM ramp during input stream) ----
        warm_w = sb_w.tile([128, 128], F16, tag="warmw")
        warm_x = sb_w.tile([128, 512], F16, tag="warmx")
        nc.vector.memset(warm_w, 0.0)
        nc.vector.memset(warm_x, 0.0)
        nc.vector.memset(ones_t, 1.0)
        # ACT table warmup (exp)
        warm_a = sb_w.tile([1, 32], F32, tag="warma")
        nc.vector.memset(warm_a, 1.0)
        warm_b = sb_w.tile([1, 32], F32, tag="warmb")
        nc.scalar.activation(out=warm_b, in_=warm_a, func=AF.Exp)

        for h in range(GH):
            z0 = 0 if h % 2 else 64
            nc.vector.memset(QT_z[h][z0:z0 + 64, :], 0.0)

        # ---- input DMAs: one priority-ordered queue (sync) ----
        nc.sync.dma_start(out=aq_t, in_=t["aq"].rearrange("p (c d) -> p c d", c=DC))
        nc.sync.dma_start(out=ak_t, in_=t["ak"].rearrange("p (c d) -> p c d", c=DC))
        nc.sync.dma_start(out=av_t, in_=t["av"].rearrange("p (c d) -> p c d", c=DC))
        def dma_x(res, src, qt):
            nc.sync.dma_start(out=res[:, qt, :],
                              in_=src[qt * 128:(qt + 1) * 128, :])

        dma_x(xq_res, t["xqb"], 0)
        dma_x(xk_res, t["xkb"], 0)
        dma_x(xv_res, t["xvb"], 0)
        for qt in range(1, QT):
            dma_x(xk_res, t["xkb"], qt)
            dma_x(xv_res, t["xvb"], qt)
            dma_x(xq_res, t["xqb"], qt)
        nc.sync.dma_start(out=bo_t, in_=t["bo"].rearrange("p (a d) -> p a d", a=2))
        if SHIFT_MODE == "pe":
            nc.sync.dma_start(out=eye_t, in_=t["eye"])
        if use_mask:
            mask_t = sb_w.tile([128, KT], F32, tag="mask")
            nc.sync.dma_start(out=mask_t, in_=t["maskf"])
        bq_t = bk_t = bv_t = None
        if use_bias:
            bq_t = sb_w.tile([1, DG], F16, tag="bq")
            bk_t = sb_w.tile([1, DG], F16, tag="bk")
            bv_t = sb_w.tile([1, DG], F16, tag="bv")
            nc.sync.dma_start(out=bq_t, in_=t["bq"])
            nc.sync.dma_start(out=bk_t, in_=t["bk"])
            nc.sync.dma_start(out=bv_t, in_=t["bv"])

        # V1 ones column (column HD of every (kt, h) slot)
        if use_mask:
            ones4 = sb_w.tile([128, GH], F32, tag="ones4")
            nc.vector.memset(ones4, 1.0)
            for kt in range(KT):
                nc.vector.tensor_scalar_mul(
                    V1[:, kt, :, HD:HD + 1],
                    ones4.rearrange("p h -> p h 1"), mask_t[:, kt:kt + 1])
        else:
            nc.vector.memset(V1[:, :, :, HD:HD + 1], 1.0)

        # ---- PE warmup: dummy matmuls to ramp HAM while inputs stream ----
        for w in range(12):
            psw = ps.tile([128, 512], F32, tag="o", bufs=2, name=f"psw_{w}")
            nc.tensor.matmul(psw[:, 0:512], warm_w, warm_x,
                             start=True, stop=True)

        # ---- emit helpers ----
        def emit_qproj(qb, p):
            # packed head pair p: one M=128 matmul per c chunk
            psq = ps.tile([128, 512], F32, tag="o", bufs=2, name=f"psq_{qb}_{p}")
            for c in range(DC):
                xsl = xq_res[:, qb, c * 512:(c + 1) * 512]
                nc.tensor.matmul(
                    psq[:, 0:512], aq_t[:, c, p * 128:(p + 1) * 128], xsl,
                    start=(c == 0), stop=(c == DC - 1 and not use_bias))
            if use_bias:
                nc.tensor.matmul(
                    psq[:, 0:512], bq_t[:, p * 128:(p + 1) * 128],
                    ones_t[0:1, :], start=False, stop=True)
            for hh in range(2):
                h = p * 2 + hh
                r0 = 64 * hh
                nc.vector.tensor_copy(
                    QT_z[h][r0:r0 + 64, qb * 512:(qb + 1) * 512],
                    psq[r0:r0 + 64, 0:512])

        def emit_kproj(qt, m):
            psm = ps.tile([128, 512], F32, tag="o", bufs=2, name=f"psk_{qt}_{m}")
            for c in range(DC):
                xsl = xk_res[:, qt, c * 512:(c + 1) * 512]
                nc.tensor.matmul(
                    psm[:, 0:512], ak_t[:, c, m * 128:(m + 1) * 128], xsl,
                    start=(c == 0), stop=(c == DC - 1 and not use_bias))
            if use_bias:
                nc.tensor.matmul(
                    psm[:, 0:512], bk_t[:, m * 128:(m + 1) * 128],
                    ones_t[0:1, :], start=False, stop=True)
            nc.vector.tensor_copy(
                KT_sb[m][:, qt * 512:(qt + 1) * 512], psm[:, 0:512])

        def emit_vproj(ktg, j):
            psv = ps.tile([128, 512], F32, tag="o", bufs=2, name=f"psv_{ktg}_{j}")
            for c in range(DC):
                xsl = xv_res[:, ktg, c * 512:(c + 1) * 512]
                nc.tensor.matmul(
                    psv[:, 0:DG], xsl[:, j * 128:(j + 1) * 128],
                    av_t[:, c, :],
                    start=(c == 0), stop=(c == DC - 1 and not use_bias))
            if use_bias:
                nc.tensor.matmul(
                    psv[:, 0:DG], ones_t[0:1, 0:128], bv_t,
                    start=False, stop=True)
            kt = ktg * 4 + j
            srcv = psv[:, 0:DG].rearrange("p (h d) -> p h d", h=GH)
            if use_mask:
                nc.vector.tensor_scalar_mul(
                    V1[:, kt, :, 0:HD], srcv, mask_t[:, kt:kt + 1])
            else:
                nc.vector.tensor_copy(V1[:, kt, :, 0:HD], srcv)

        # ---- the attention round engine ----
        # round r = (qb, hp, sk, hh): S^T (2 MMs -> pss), exp, then PV of
        # the PREVIOUS round (software pipeline, 1-round lag).
        filler = []           # list of closures, each ~2 matmuls
        fill_debt = [0.0]     # fractional chunks owed

        def pop_filler(n=1.0):
            fill_debt[0] += n
            while fill_debt[0] >= 1.0 and filler:
                filler.pop(0)()
                fill_debt[0] -= 1.0

        def emit_st(qb, hp, sk, hh):
            h = hp * 2 + hh
            pss = ps.tile([128, 1024], F32, tag="s", bufs=2,
                          name=f"pss_{qb}_{sk}_{h}")
            for dk in range(2):
                kt = sk * 2 + dk
                nc.tensor.matmul(
                    pss[:, dk * 512:(dk + 1) * 512],
                    KT_sb[hp][:, kt * 128:(kt + 1) * 128],
                    QT_z[h][:, qb * 512:qb * 512 + QB],
                    start=True, stop=True)
            pt = sb_pt.tile([128, 1024], F16, tag="pt",
                            name=f"pt_{qb}_{sk}_{h}")
            nc.scalar.activation(out=pt, in_=pss, func=AF.Exp, scale=SCALE)
            return pt

        def emit_pv(qb, hp, sk, hh, pt, pso):
            h = hp * 2 + hh
            for dk in range(2):
                kt = sk * 2 + dk
                nc.tensor.matmul(
                    pso[hh][0:HD + 1, :], V1[:, kt, h, :],
                    pt[:, dk * 512:(dk + 1) * 512],
                    start=(kt == 0), stop=(kt == KT - 1))

        def emit_hp_tail(qb, hp, pso):
            # normalize both heads of the pair into oN2[qb%2][hp]
            on = oN2[qb % 2][hp]
            for hh in range(2):
                oT = sb_n.tile([HD + 1, 512], F16, tag="oT", bufs=4,
                               name=f"oT_{qb}_{hp}_{hh}")
                nc.vector.tensor_copy(oT, pso[hh][0:HD + 1, :])
                bc = ps.tile([128, 512], F32, tag="o", bufs=2,
                             name=f"bc_{qb}_{hp}_{hh}")
                nc.tensor.matmul(
                    bc[0:64, :], ones_t[64:65, 0:64], oT[64:65, :],
                    start=True, stop=True, tile_position=(64, 0))
                rcp = sb_n.tile([64, 512], F32, tag="rcp", bufs=2,
                                name=f"rcp_{qb}_{hp}_{hh}")
                nc.vector.reciprocal(out=rcp, in_=bc[0:64, :])
                if hh == 0:
                    nc.vector.tensor_tensor(on[0:64, :], oT[0:64, :], rcp,
                                            op=MULT)
                elif SHIFT_MODE == "dve":
                    nc.vector.tensor_tensor(on[64:128, :], oT[0:64, :], rcp,
                                            op=MULT)
                else:
                    tmp = sb_n.tile([64, 512], F16, tag="tmp", bufs=2,
                                    name=f"tmp_{qb}_{hp}")
                    nc.vector.tensor_tensor(tmp, oT[0:64, :], rcp, op=MULT)
                    sh = ps.tile([128, 512], F32, tag="o", bufs=2,
                                 name=f"sh_{qb}_{hp}")
                    nc.tensor.matmul(sh[64:128, :], eye_t, tmp,
                                     start=True, stop=True,
                                     tile_position=(0, 64))
                    nc.vector.tensor_copy(on[64:128, :], sh[64:128, :])

        def emit_outproj_chunk(qb, mq, nb, psout_box):
            # two packed MMs (hp 0,1) accumulating psout, then CAST out
            on_pair = oN2[qb % 2]
            psout = ps.tile([128, 512], F32, tag="o", bufs=2,
                            name=f"psout_{qb}_{mq}_{nb}")
            for hp in range(2):
                nc.tensor.matmul(
                    psout[:, 0:512],
                    on_pair[hp][:, mq * 128:(mq + 1) * 128],
                    bo_t[:, hp, nb * 512:(nb + 1) * 512],
                    start=(hp == 0), stop=(hp == 1))
            psout_box[nb] = psout

        def emit_outproj(qb):
            # returns filler closures: 8 chunks of 2 MMs + CAST/DMA
            chunks = []
            for mq in range(4):
                ot = sb_out.tile([128, D], F16, tag="ot", name=f"ot_{qb}_{mq}")
                box = {}

                def mk(qb=qb, mq=mq, ot=ot, box=box):
                    def c0():
                        emit_outproj_chunk(qb, mq, 0, box)
                        nc.vector.tensor_copy(ot[:, 0:512], box[0][:, 0:512])

                    def c1():
                        emit_outproj_chunk(qb, mq, 1, box)
                        nc.vector.tensor_copy(ot[:, 512:1024], box[1][:, 0:512])
                        q0 = qb * QB + mq * 128
                        nc.gpsimd.dma_start(out=t["outp"][q0:q0 + 128, :],
                                            in_=ot)
                    return [c0, c1]
                chunks.extend(mk())
            return chunks

        # ---- schedule ----
        emit_qproj(0, 0)
        emit_qproj(0, 1)
        emit_kproj(0, 0)
        emit_kproj(0, 1)
        for j in range(4):
            emit_vproj(0, j)

        # qb0/hp0 filler map (round index -> closures), deadline-correct:
        # kproj(qt,0) must be EMITTED before the S^T of sk=2qt (round 4qt);
        # vproj(qt,j) before the PV of sk covering kt=4qt+j (PV lags one
        # round); kproj(*,1) is only needed by hp1's rounds.
        qb0_sched = {
            0: [lambda: emit_kproj(1, 0)],
            1: [lambda: emit_vproj(1, 0), lambda: emit_vproj(1, 1)],
            2: [lambda: emit_vproj(1, 2), lambda: emit_vproj(1, 3)],
            3: [lambda: emit_kproj(2, 0)],
            4: [lambda: emit_vproj(2, 0), lambda: emit_vproj(2, 1)],
            5: [lambda: emit_vproj(2, 2), lambda: emit_vproj(2, 3)],
            6: [lambda: emit_kproj(3, 0)],
            7: [lambda: emit_vproj(3, 0), lambda: emit_vproj(3, 1)],
            8: [lambda: emit_vproj(3, 2), lambda: emit_vproj(3, 3)],
            9: [lambda: emit_kproj(1, 1)],
            10: [lambda: emit_kproj(2, 1)],
            11: [lambda: emit_kproj(3, 1)],
            12: [lambda: emit_qproj(1, 0)],
            13: [lambda: emit_qproj(1, 1)],
        }

        def run_qb(qb):
            for hp in range(2):
                pso = [ps.tile([128, 512], F32, tag="acc", bufs=2,
                               name=f"pso_{qb}_{hp}_{hh}") for hh in range(2)]
                prev = None
                ridx = 0
                for sk in range(8):
                    for hh in range(2):
                        if qb == 0 and hp == 0:
                            for fn in qb0_sched.pop(ridx, []):
                                fn()
                        pt = emit_st(qb, hp, sk, hh)
                        if prev is not None:
                            emit_pv(*prev)
                        if not (qb == 0 and hp == 0):
                            pop_filler(1.0)
                        prev = (qb, hp, sk, hh, pt, pso)
                        ridx += 1
                emit_pv(*prev)
                emit_hp_tail(qb, hp, pso)

        run_qb(0)
        for ri in sorted(qb0_sched):
            for fn in qb0_sched.pop(ri, []):
                fn()
        for qb in range(1, QT):
            filler.extend(emit_outproj(qb - 1))
            if qb + 1 < QT:
                filler.append(lambda qb=qb: emit_qproj(qb + 1, 0))
                filler.append(lambda qb=qb: emit_qproj(qb + 1, 1))
            run_qb(qb)
            while filler:
                filler.pop(0)()
        for ch in emit_outproj(QT - 1):
            ch()


def _swizzle_a(aT):
    """[D, DG] -> [128, DC*DG]: partition p holds chunks c at (c, :)."""
    return np.ascontiguousarray(
        aT.reshape(DC, 128, DG).transpose(1, 0, 2).reshape(128, DC * DG))


def _pack_bo(boT):
    """[DG, D] -> [128, 2*D]: head-pair hp at cols hp*D, rows=pair dims."""
    out = boT.reshape(2, 128, D).transpose(1, 0, 2)
    return np.ascontiguousarray(out.reshape(128, 2 * D))


def _block_x(xT):
    """[D, L] -> [QT*128, DC*512] qt-major blocks, 8KB partition lines."""
    return np.ascontiguousarray(
        xT.reshape(DC, 128, QT, 512).transpose(2, 1, 0, 3).reshape(
            QT * 128, DC * 512))


_EYE = np.eye(64, dtype=np.float16)


def _prep_inputs(values, key, query, mask, Wv, Wk, Wq, Wo, bv, bk, bq):
    """Build the 8 per-core input maps (host-side shard + layout)."""
    xB = {}
    for n in range(N_BATCH):
        xB[("q", n)] = _block_x(query[n].T.astype(np.float16))
        xB[("k", n)] = _block_x(key[n].T.astype(np.float16))
        xB[("v", n)] = _block_x(values[n].T.astype(np.float16))
    in_maps = []
    for c in range(CORES):
        n, g = divmod(c, CORES // N_BATCH)
        rows = slice(g * DG, (g + 1) * DG)
        mrow = np.ascontiguousarray(
            mask[n, 0, 0, :].astype(np.float32).reshape(KT, 128).T)
        in_maps.append({
            "xqb": xB[("q", n)],
            "xkb": xB[("k", n)],
            "xvb": xB[("v", n)],
            "aq": _swizzle_a(Wq[rows, :].T.astype(np.float16)),
            "ak": _swizzle_a(Wk[rows, :].T.astype(np.float16)),
            "av": _swizzle_a(Wv[rows, :].T.astype(np.float16)),
            "bo": _pack_bo(Wo[:, rows].T.astype(np.float16)),
            "bq": np.ascontiguousarray(bq[None, rows].astype(np.float16)),
            "bk": np.ascontiguousarray(bk[None, rows].astype(np.float16)),
            "bv": np.ascontiguousarray(bv[None, rows].astype(np.float16)),
            "eye": _EYE,
            "maskf": mrow,
        })
    return in_maps


LAST_EXEC_NS = None
LAST_RES = None


def kernel(values, key, query, mask, Wv, bv, Wk, bk, Wq, bq, Wo, bo,
           trace=False, trace_cores=None):
    global LAST_EXEC_NS, LAST_RES
    values = np.asarray(values, dtype=np.float32)
    key = np.asarray(key, dtype=np.float32)
    query = np.asarray(query, dtype=np.float32)
    mask = np.asarray(mask)
    Wq, Wk, Wv, Wo = (np.asarray(Wq, np.float32), np.asarray(Wk, np.float32),
                      np.asarray(Wv, np.float32), np.asarray(Wo, np.float32))
    bq, bk, bv, bo = (np.asarray(bq, np.float32), np.asarray(bk, np.float32),
                      np.asarray(bv, np.float32), np.asarray(bo, np.float32))

    use_bias = bool(np.any(bq) or np.any(bk) or np.any(bv))
    use_mask = not bool(np.all(np.asarray(mask) == 1))

    nc = _build(use_bias, use_mask)
    in_maps = _prep_inputs(values, key, query, mask, Wv, Wk, Wq, Wo,
                           bv, bk, bq)
    kw = {}
    if trace_cores is not None:
        kw["trace_cores"] = trace_cores
    res = run_bass_kernel_spmd(nc, in_maps, core_ids=list(range(CORES)),
                               trace=trace, **kw)
    LAST_EXEC_NS = res.exec_time_ns
    LAST_RES = res

    out = np.zeros((N_BATCH, L, D), dtype=np.float32)
    for c in range(CORES):
        n = c // (CORES // N_BATCH)
        out[n] += res.results[c]["outp"].astype(np.float32)
    out += bo[None, None, :]
    return out


# revision 11
# speedup vs baseline: 1.1782x; 1.1719x over previous
"""Multi-head attention (N=2, L=2048, D=1024, H=16) on 8 NeuronCores.

Sharding: core c -> (batch n = c // 4, head group g = c % 4, 4 heads each).
Each core computes Q/K/V projections for its 4 heads, attention, and its
slice of the output projection. Host sums the 4 partial output projections
per batch and adds bo.

v2 design (from baseline profiling: PE 210.7us busy, exec 260us, 31.5us
PE gaps + ~25us HAM cold-clock tax + 12.4us tail):
- Host-blocked qt-major x layouts ([QT, 128, DC*512], 8KB DMA lines) so
  inputs arrive in consumption order; kproj(0)/vproj(0)/qproj(0) feed at
  ~13-19us and the attention pipeline ignites at ~22us.
- Single-head rounds: per (qb, hp, sk, hh): S^T = 2 MMs into one
  [128,1024] fp32 psum tile (ring 2), one exp (ScalarE, scale=1/32),
  PV = 2 MMs (M=65: V dims + ones column accumulating the softmax
  denominator). Ring granularity == round granularity so the exp stream
  never stalls on psum (baseline allocated 4 tiles/round vs ring 2).
- Head-pair outer loop (hp): only 2 PV accumulators live -> PSUM fits:
  s(2x2 banks) + acc(2x1) + o(2x1) = 8 banks, leaving an "o" ring for
  interleaved filler matmuls (projections + prev-qb out-proj).
- Out-proj packs head pairs: oN2[hp] [128,512] holds both heads' dims ->
  K=128 fully used, 8 MMs/qb instead of 16. Odd head's normalized rows
  are written at partition offset 64 (SHIFT_MODE selects DVE direct
  offset write vs PE identity-shift matmul).
- Reciprocal on DVE: bc matmul broadcasts the RAW denominator row (K=1
  matmul from partition 64), nc.vector.reciprocal on [64,512], then one
  fused tensor_tensor multiply -> ScalarE runs exps only (no Ln/Exp
  pair, no act-table patching).
- PE warmup matmuls on memset tiles during the DMA lead-in (HAM).
"""
import os
import sys
import types

import numpy as np

N_BATCH = 2
L = 2048
D = 1024
H = 16
HD = 64
CORES = 8
GH = 4            # heads per core
DG = GH * HD      # 256 = projected dims per core
QB = 512          # q block
KT = L // 128     # 16 k tiles
QT = L // QB      # 4 q blocks
DC = D // 128     # 8 din chunks
SCALE = 1.0 / 32.0  # 1/sqrt(D)
SHIFT_MODE = "dve"  # "dve": direct offset write; "pe": identity matmul shift


def _install_ntff_hook():
    """The image's antenv stub lacks axon_hooks; shim it so trace=True works."""
    if "antenv.axon_hooks" in sys.modules:
        return
    mod = types.ModuleType("antenv.axon_hooks")
    mod._hook = None
    mod.set_axon_ntff_profile_hook = lambda h: setattr(mod, "_hook", h)
    mod.get_axon_ntff_profile_hook = lambda: mod._hook
    sys.modules["antenv.axon_hooks"] = mod
    try:
        from trn_agent_boot.trn_boot import _ntff_profile_via_ctypes
        mod._hook = _ntff_profile_via_ctypes("/opt/axon/libaxon_pjrt.so")
    except Exception:
        mod._hook = None


_install_ntff_hook()

import concourse.bacc as bacc
import concourse.mybir as mybir
import concourse.tile as tile
from concourse.bass_utils import run_bass_kernel_spmd

F32 = mybir.dt.float32
F16 = mybir.dt.float16
AF = mybir.ActivationFunctionType
MULT = mybir.AluOpType.mult

_CACHE = {}


def _build(use_bias, use_mask):
    key = (use_bias, use_mask)
    if key in _CACHE:
        return _CACHE[key]

    nc = bacc.Bacc("TRN2", debug=False, num_devices=CORES)

    xqb = nc.dram_tensor("xqb", [QT * 128, DC * 512], F16, kind="ExternalInput").ap()
    xkb = nc.dram_tensor("xkb", [QT * 128, DC * 512], F16, kind="ExternalInput").ap()
    xvb = nc.dram_tensor("xvb", [QT * 128, DC * 512], F16, kind="ExternalInput").ap()
    aq = nc.dram_tensor("aq", [128, DC * DG], F16, kind="ExternalInput").ap()
    ak = nc.dram_tensor("ak", [128, DC * DG], F16, kind="ExternalInput").ap()
    av = nc.dram_tensor("av", [128, DC * DG], F16, kind="ExternalInput").ap()
    bo = nc.dram_tensor("bo", [128, 2 * D], F16, kind="ExternalInput").ap()
    bq = nc.dram_tensor("bq", [1, DG], F16, kind="ExternalInput").ap()
    bk = nc.dram_tensor("bk", [1, DG], F16, kind="ExternalInput").ap()
    bv = nc.dram_tensor("bv", [1, DG], F16, kind="ExternalInput").ap()
    eye = nc.dram_tensor("eye", [64, 64], F16, kind="ExternalInput").ap()
    maskf = nc.dram_tensor("maskf", [128, KT], F32, kind="ExternalInput").ap()
    outp = nc.dram_tensor("outp", [L, D], F16, kind="ExternalOutput").ap()

    with tile.TileContext(nc) as tc:
        _emit(nc, tc, dict(xqb=xqb, xkb=xkb, xvb=xvb, aq=aq, ak=ak, av=av,
                           bo=bo, bq=bq, bk=bk, bv=bv, eye=eye, maskf=maskf,
                           outp=outp),
              use_bias, use_mask)
    nc.compile()
    _CACHE[key] = nc
    return nc


def _emit(nc, tc, t, use_bias, use_mask):
    from contextlib import ExitStack
    ctx = ExitStack()
    with ctx:
        sb_w = ctx.enter_context(tc.tile_pool(name="sb_w", bufs=1))
        sb_qkv = ctx.enter_context(tc.tile_pool(name="sb_qkv", bufs=1))
        sb_pt = ctx.enter_context(tc.tile_pool(name="sb_pt", bufs=4))
        sb_n = ctx.enter_context(tc.tile_pool(name="sb_n", bufs=4))
        sb_out = ctx.enter_context(tc.tile_pool(name="sb_out", bufs=3))
        ps = ctx.enter_context(tc.tile_pool(name="ps", bufs=2, space="PSUM"))

        # ---- resident tiles ----
        aq_t = sb_w.tile([128, DC, DG], F16, tag="aq")
        ak_t = sb_w.tile([128, DC, DG], F16, tag="ak")
        av_t = sb_w.tile([128, DC, DG], F16, tag="av")
        bo_t = sb_w.tile([128, 2, D], F16, tag="bo")
        ones_t = sb_w.tile([128, 512], F16, tag="ones")
        eye_t = sb_w.tile([64, 64], F16, tag="eye")
        xq_res = sb_w.tile([128, QT, DC * 512], F16, tag="xq")
        xk_res = sb_w.tile([128, QT, DC * 512], F16, tag="xk")
        xv_res = sb_w.tile([128, QT, DC * 512], F16, tag="xv")
        KT_sb = [sb_qkv.tile([128, L], F16, tag=f"kt{m}", name=f"KTm{m}")
                 for m in range(2)]
        QT_z = [sb_qkv.tile([128, L], F16, tag=f"qz{h}", name=f"QTz{h}")
                for h in range(GH)]
        V1 = sb_qkv.tile([128, KT, GH, HD + 1], F16, tag="v1")
        # oN2[qb%2][hp]: packed normalized heads for the out-projection
        oN2 = [[sb_qkv.tile([128, 512], F16, tag=f"oN{b}{hp}",
                            name=f"oN{b}{hp}") for hp in range(2)]
               for b in range(2)]

        # ---- warmup tiles (no DMA deps; HAM ramp during input stream) ----
        warm_w = sb_w.tile([128, 128], F16, tag="warmw")
        warm_x = sb_w.tile([128, 512], F16, tag="warmx")
        nc.vector.memset(warm_w, 0.0)
        nc.vector.memset(warm_x, 0.0)
        nc.vector.memset(ones_t, 1.0)
        # ACT table warmup (exp)
        warm_a = sb_w.tile([1, 32], F32, tag="warma")
        nc.vector.memset(warm_a, 1.0)
        warm_b = sb_w.tile([1, 32], F32, tag="warmb")
        nc.scalar.activation(out=warm_b, in_=warm_a, func=AF.Exp)

        for h in range(GH):
            z0 = 0 if h % 2 else 64
            nc.vector.memset(QT_z[h][z0:z0 + 64, :], 0.0)

        # ---- input DMAs: one priority-ordered queue (sync) ----
        # First tiles split in halves so qproj/kproj start ~2us earlier.
        def dma_x(res, src, qt, half=None):
            if half is None:
                nc.sync.dma_start(out=res[:, qt, :],
                                  in_=src[qt * 128:(qt + 1) * 128, :])
            else:
                h0 = half * (DC // 2) * 512
                h1 = h0 + (DC // 2) * 512
                nc.sync.dma_start(out=res[:, qt, h0:h1],
                                  in_=src[qt * 128:(qt + 1) * 128, h0:h1])

        nc.sync.dma_start(out=aq_t, in_=t["aq"].rearrange("p (c d) -> p c d", c=DC))
        dma_x(xq_res, t["xqb"], 0, 0)
        dma_x(xq_res, t["xqb"], 0, 1)
        nc.sync.dma_start(out=ak_t, in_=t["ak"].rearrange("p (c d) -> p c d", c=DC))
        dma_x(xk_res, t["xkb"], 0, 0)
        dma_x(xk_res, t["xkb"], 0, 1)
        nc.sync.dma_start(out=av_t, in_=t["av"].rearrange("p (c d) -> p c d", c=DC))
        dma_x(xv_res, t["xvb"], 0, 0)
        dma_x(xv_res, t["xvb"], 0, 1)
        for qt in range(1, QT):
            dma_x(xk_res, t["xkb"], qt)
            dma_x(xv_res, t["xvb"], qt)
            dma_x(xq_res, t["xqb"], qt)
        nc.sync.dma_start(out=bo_t, in_=t["bo"].rearrange("p (a d) -> p a d", a=2))
        if SHIFT_MODE == "pe":
            nc.sync.dma_start(out=eye_t, in_=t["eye"])
        if use_mask:
            mask_t = sb_w.tile([128, KT], F32, tag="mask")
            nc.sync.dma_start(out=mask_t, in_=t["maskf"])
        bq_t = bk_t = bv_t = None
        if use_bias:
            bq_t = sb_w.tile([1, DG], F16, tag="bq")
            bk_t = sb_w.tile([1, DG], F16, tag="bk")
            bv_t = sb_w.tile([1, DG], F16, tag="bv")
            nc.sync.dma_start(out=bq_t, in_=t["bq"])
            nc.sync.dma_start(out=bk_t, in_=t["bk"])
            nc.sync.dma_start(out=bv_t, in_=t["bv"])

        # V1 ones column (column HD of every (kt, h) slot)
        if use_mask:
            ones4 = sb_w.tile([128, GH], F32, tag="ones4")
            nc.vector.memset(ones4, 1.0)
            for kt in range(KT):
                nc.vector.tensor_scalar_mul(
                    V1[:, kt, :, HD:HD + 1],
                    ones4.rearrange("p h -> p h 1"), mask_t[:, kt:kt + 1])
        else:
            nc.vector.memset(V1[:, :, :, HD:HD + 1], 1.0)

        # ---- PE warmup: dummy matmuls to ramp HAM while inputs stream ----
        for w in range(14):
            psw = ps.tile([128, 512], F32, tag="o", bufs=2, name=f"psw_{w}")
            nc.tensor.matmul(psw[:, 0:512], warm_w, warm_x,
                             start=True, stop=True)

        # ---- emit helpers ----
        def emit_qproj(qb, p):
            # packed head pair p: one M=128 matmul per c chunk
            psq = ps.tile([128, 512], F32, tag="o", bufs=2, name=f"psq_{qb}_{p}")
            for c in range(DC):
                xsl = xq_res[:, qb, c * 512:(c + 1) * 512]
                nc.tensor.matmul(
                    psq[:, 0:512], aq_t[:, c, p * 128:(p + 1) * 128], xsl,
                    start=(c == 0), stop=(c == DC - 1 and not use_bias))
            if use_bias:
                nc.tensor.matmul(
                    psq[:, 0:512], bq_t[:, p * 128:(p + 1) * 128],
                    ones_t[0:1, :], start=False, stop=True)
            for hh in range(2):
                h = p * 2 + hh
                r0 = 64 * hh
                nc.vector.tensor_copy(
                    QT_z[h][r0:r0 + 64, qb * 512:(qb + 1) * 512],
                    psq[r0:r0 + 64, 0:512])

        def emit_kproj(qt, m):
            psm = ps.tile([128, 512], F32, tag="o", bufs=2, name=f"psk_{qt}_{m}")
            for c in range(DC):
                xsl = xk_res[:, qt, c * 512:(c + 1) * 512]
                nc.tensor.matmul(
                    psm[:, 0:512], ak_t[:, c, m * 128:(m + 1) * 128], xsl,
                    start=(c == 0), stop=(c == DC - 1 and not use_bias))
            if use_bias:
                nc.tensor.matmul(
                    psm[:, 0:512], bk_t[:, m * 128:(m + 1) * 128],
                    ones_t[0:1, :], start=False, stop=True)
            nc.vector.tensor_copy(
                KT_sb[m][:, qt * 512:(qt + 1) * 512], psm[:, 0:512])

        def emit_vproj(ktg, j):
            psv = ps.tile([128, 512], F32, tag="o", bufs=2, name=f"psv_{ktg}_{j}")
            for c in range(DC):
                xsl = xv_res[:, ktg, c * 512:(c + 1) * 512]
                nc.tensor.matmul(
                    psv[:, 0:DG], xsl[:, j * 128:(j + 1) * 128],
                    av_t[:, c, :],
                    start=(c == 0), stop=(c == DC - 1 and not use_bias))
            if use_bias:
                nc.tensor.matmul(
                    psv[:, 0:DG], ones_t[0:1, 0:128], bv_t,
                    start=False, stop=True)
            kt = ktg * 4 + j
            srcv = psv[:, 0:DG].rearrange("p (h d) -> p h d", h=GH)
            if use_mask:
                nc.vector.tensor_scalar_mul(
                    V1[:, kt, :, 0:HD], srcv, mask_t[:, kt:kt + 1])
            else:
                nc.vector.tensor_copy(V1[:, kt, :, 0:HD], srcv)

        # ---- the attention round engine ----
        # round r = (qb, hp, sk, hh): S^T (2 MMs -> pss), exp, then PV of
        # the PREVIOUS round (software pipeline, 1-round lag).
        filler = []           # list of closures, each ~2 matmuls
        fill_debt = [0.0]     # fractional chunks owed

        def pop_filler(n=1.0):
            fill_debt[0] += n
            while fill_debt[0] >= 1.0 and filler:
                filler.pop(0)()
                fill_debt[0] -= 1.0

        def emit_st(qb, hp, sk, hh):
            h = hp * 2 + hh
            pss = ps.tile([128, 1024], F32, tag="s", bufs=2,
                          name=f"pss_{qb}_{sk}_{h}")
            for dk in range(2):
                kt = sk * 2 + dk
                nc.tensor.matmul(
                    pss[:, dk * 512:(dk + 1) * 512],
                    KT_sb[hp][:, kt * 128:(kt + 1) * 128],
                    QT_z[h][:, qb * 512:qb * 512 + QB],
                    start=True, stop=True)
            pt = sb_pt.tile([128, 1024], F16, tag="pt",
                            name=f"pt_{qb}_{sk}_{h}")
            nc.scalar.activation(out=pt, in_=pss, func=AF.Exp, scale=SCALE)
            return pt

        def emit_pv(qb, hp, sk, hh, pt, pso):
            h = hp * 2 + hh
            for dk in range(2):
                kt = sk * 2 + dk
                nc.tensor.matmul(
                    pso[hh][0:HD + 1, :], V1[:, kt, h, :],
                    pt[:, dk * 512:(dk + 1) * 512],
                    start=(kt == 0), stop=(kt == KT - 1))

        def emit_hp_tail(qb, hp, pso):
            # normalize both heads of the pair into oN2[qb%2][hp];
            # the two heads' chains are interleaved so engines pipeline
            on = oN2[qb % 2][hp]
            oTs, bcs, rcps = [], [], []
            for hh in range(2):
                oT = sb_n.tile([HD + 1, 512], F16, tag="oT", bufs=4,
                               name=f"oT_{qb}_{hp}_{hh}")
                nc.vector.tensor_copy(oT, pso[hh][0:HD + 1, :])
                oTs.append(oT)
            for hh in range(2):
                bc = ps.tile([128, 512], F32, tag="o", bufs=2,
                             name=f"bc_{qb}_{hp}_{hh}")
                nc.tensor.matmul(
                    bc[0:64, :], ones_t[64:65, 0:64], oTs[hh][64:65, :],
                    start=True, stop=True, tile_position=(64, 0))
                bcs.append(bc)
            for hh in range(2):
                rcp = sb_n.tile([64, 512], F32, tag="rcp", bufs=2,
                                name=f"rcp_{qb}_{hp}_{hh}")
                nc.vector.reciprocal_approx_fast(out=rcp, in_=bcs[hh][0:64, :])
                rcps.append(rcp)
            for hh in range(2):
                rows = slice(0, 64) if hh == 0 else slice(64, 128)
                nc.vector.tensor_tensor(on[rows, :], oTs[hh][0:64, :],
                                        rcps[hh], op=MULT)

        def emit_outproj_chunk(qb, mq, nb, psout_box, tag="o"):
            # two packed MMs (hp 0,1) accumulating psout, then CAST out
            on_pair = oN2[qb % 2]
            psout = ps.tile([128, 512], F32, tag=tag, bufs=2,
                            name=f"psout_{qb}_{mq}_{nb}")
            for hp in range(2):
                nc.tensor.matmul(
                    psout[:, 0:512],
                    on_pair[hp][:, mq * 128:(mq + 1) * 128],
                    bo_t[:, hp, nb * 512:(nb + 1) * 512],
                    start=(hp == 0), stop=(hp == 1))
            psout_box[nb] = psout

        def emit_outproj(qb, deep=False):
            # returns filler closures: 8 chunks of 2 MMs + CAST/DMA.
            # deep=True (endgame): alternate psum tags "o"/"s" so 4 psout
            # tiles pipeline (the attention rings are drained by then).
            chunks = []
            for mq in range(4):
                ot = sb_out.tile([128, D], F16, tag="ot", name=f"ot_{qb}_{mq}")
                box = {}
                tg0 = "s" if deep and mq % 2 else "o"
                tg1 = "s" if deep and not mq % 2 else "o"

                def mk(qb=qb, mq=mq, ot=ot, box=box, tg0=tg0, tg1=tg1):
                    def c0():
                        emit_outproj_chunk(qb, mq, 0, box, tg0)
                        nc.vector.tensor_copy(ot[:, 0:512], box[0][:, 0:512])

                    def c1():
                        emit_outproj_chunk(qb, mq, 1, box, tg1)
                        nc.vector.tensor_copy(ot[:, 512:1024], box[1][:, 0:512])
                        q0 = qb * QB + mq * 128
                        nc.gpsimd.dma_start(out=t["outp"][q0:q0 + 128, :],
                                            in_=ot)
                    return [c0, c1]
                chunks.extend(mk())
            return chunks

        # ---- schedule ----
        emit_qproj(0, 0)
        emit_qproj(0, 1)
        emit_kproj(0, 0)
        emit_kproj(0, 1)
        for j in range(4):
            emit_vproj(0, j)

        # qb0 filler maps (round index -> closures), deadline-correct:
        # kproj(qt,0) must be EMITTED before hp0's S^T of sk=2qt (round
        # 4qt... ridx 2sk+hh); vproj(qt,j) before the PV of the sk
        # covering kt (PV lags one round); kproj(*,1) before hp1's rounds.
        sched_q0h0 = {
            0: [lambda: emit_kproj(1, 0)],
            1: [lambda: emit_vproj(1, 0)],
            2: [lambda: emit_vproj(1, 1)],
            3: [lambda: emit_vproj(1, 2), lambda: emit_vproj(1, 3)],
            4: [lambda: emit_kproj(2, 0)],
            5: [lambda: emit_vproj(2, 0)],
            6: [lambda: emit_vproj(2, 1)],
            7: [lambda: emit_vproj(2, 2), lambda: emit_vproj(2, 3)],
            8: [lambda: emit_kproj(3, 0)],
            9: [lambda: emit_vproj(3, 0)],
            10: [lambda: emit_vproj(3, 1)],
            11: [lambda: emit_vproj(3, 2), lambda: emit_vproj(3, 3)],
            12: [lambda: emit_kproj(1, 1)],
            13: [lambda: emit_kproj(2, 1)],
            14: [lambda: emit_kproj(3, 1)],
        }
        sched_q0h1 = {
            0: [lambda: emit_qproj(1, 1)],
            6: [lambda: emit_qproj(2, 0)],
        }

        def spread(items, n_rounds=32, reserve=2):
            """Assign items evenly to round indices, keeping `reserve`
            items back for the hp-tail boundaries."""
            body = items[:len(items) - reserve] if reserve else items
            tail = items[len(items) - reserve:] if reserve else []
            m = {}
            if body:
                for i, it in enumerate(body):
                    m.setdefault(i * n_rounds // len(body), []).append(it)
            return m, tail

        def run_qb(qb, s0, s1, t0, t1):
            for hp in range(2):
                sched = s0 if hp == 0 else s1
                tailf = t0 if hp == 0 else t1
                pso = [ps.tile([128, 512], F32, tag="acc", bufs=2,
                               name=f"pso_{qb}_{hp}_{hh}") for hh in range(2)]
                prev = None
                ridx = 0
                for sk in range(8):
                    for hh in range(2):
                        for fn in sched.pop(ridx, []):
                            fn()
                        pt = emit_st(qb, hp, sk, hh)
                        if prev is not None:
                            emit_pv(*prev)
                        prev = (qb, hp, sk, hh, pt, pso)
                        ridx += 1
                emit_pv(*prev)
                for fn in tailf:
                    fn()
                emit_hp_tail(qb, hp, pso)

        run_qb(0, sched_q0h0, sched_q0h1,
               [lambda: emit_qproj(1, 0)], [lambda: emit_qproj(2, 1)])
        for qb in range(1, QT):
            items = emit_outproj(qb - 1)
            if qb == 1:
                items += [lambda: emit_qproj(3, 0), lambda: emit_qproj(3, 1)]
            half0, rest = items[:len(items) // 2], items[len(items) // 2:]
            s0, t0 = spread(half0, n_rounds=16, reserve=1)
            s1, t1 = spread(rest, n_rounds=16, reserve=1)
            run_qb(qb, s0, s1, t0, t1)
        for ch in emit_outproj(QT - 1, deep=True):
            ch()


def _swizzle_a(aT):
    """[D, DG] -> [128, DC*DG]: partition p holds chunks c at (c, :)."""
    return np.ascontiguousarray(
        aT.reshape(DC, 128, DG).transpose(1, 0, 2).reshape(128, DC * DG))


def _pack_bo(boT):
    """[DG, D] -> [128, 2*D]: head-pair hp at cols hp*D, rows=pair dims."""
    out = boT.reshape(2, 128, D).transpose(1, 0, 2)
    return np.ascontiguousarray(out.reshape(128, 2 * D))


def _block_x(xT):
    """[D, L] -> [QT*128, DC*512] qt-major blocks, 8KB partition lines."""
    return np.ascontiguousarray(
        xT.reshape(DC, 128, QT, 512).transpose(2, 1, 0, 3).reshape(
            QT * 128, DC * 512))


_EYE = np.eye(64, dtype=np.float16)


def _prep_inputs(values, key, query, mask, Wv, Wk, Wq, Wo, bv, bk, bq):
    """Build the 8 per-core input maps (host-side shard + layout)."""
    xB = {}
    for n in range(N_BATCH):
        xB[("q", n)] = _block_x(query[n].T.astype(np.float16))
        xB[("k", n)] = _block_x(key[n].T.astype(np.float16))
        xB[("v", n)] = _block_x(values[n].T.astype(np.float16))
    in_maps = []
    for c in range(CORES):
        n, g = divmod(c, CORES // N_BATCH)
        rows = slice(g * DG, (g + 1) * DG)
        mrow = np.ascontiguousarray(
            mask[n, 0, 0, :].astype(np.float32).reshape(KT, 128).T)
        in_maps.append({
            "xqb": xB[("q", n)],
            "xkb": xB[("k", n)],
            "xvb": xB[("v", n)],
            "aq": _swizzle_a(Wq[rows, :].T.astype(np.float16)),
            "ak": _swizzle_a(Wk[rows, :].T.astype(np.float16)),
            "av": _swizzle_a(Wv[rows, :].T.astype(np.float16)),
            "bo": _pack_bo(Wo[:, rows].T.astype(np.float16)),
            "bq": np.ascontiguousarray(bq[None, rows].astype(np.float16)),
            "bk": np.ascontiguousarray(bk[None, rows].astype(np.float16)),
            "bv": np.ascontiguousarray(bv[None, rows].astype(np.float16)),
            "eye": _EYE,
            "maskf": mrow,
        })
    return in_maps


LAST_EXEC_NS = None
LAST_RES = None


def kernel(values, key, query, mask, Wv, bv, Wk, bk, Wq, bq, Wo, bo,
           trace=False, trace_cores=None):
    global LAST_EXEC_NS, LAST_RES
    values = np.asarray(values, dtype=np.float32)
    key = np.asarray(key, dtype=np.float32)
    query = np.asarray(query, dtype=np.float32)
    mask = np.asarray(mask)
    Wq, Wk, Wv, Wo = (np.asarray(Wq, np.float32), np.asarray(Wk, np.float32),
                      np.asarray(Wv, np.float32), np.asarray(Wo, np.float32))
    bq, bk, bv, bo = (np.asarray(bq, np.float32), np.asarray(bk, np.float32),
                      np.asarray(bv, np.float32), np.asarray(bo, np.float32))

    use_bias = bool(np.any(bq) or np.any(bk) or np.any(bv))
    use_mask = not bool(np.all(np.asarray(mask) == 1))

    nc = _build(use_bias, use_mask)
    in_maps = _prep_inputs(values, key, query, mask, Wv, Wk, Wq, Wo,
                           bv, bk, bq)
    kw = {}
    if trace_cores is not None:
        kw["trace_cores"] = trace_cores
    res = run_bass_kernel_spmd(nc, in_maps, core_ids=list(range(CORES)),
                               trace=trace, **kw)
    LAST_EXEC_NS = res.exec_time_ns
    LAST_RES = res

    out = np.zeros((N_BATCH, L, D), dtype=np.float32)
    for c in range(CORES):
        n = c // (CORES // N_BATCH)
        out[n] += res.results[c]["outp"].astype(np.float32)
    out += bo[None, None, :]
    return out


# revision 14
# speedup vs baseline: 1.1917x; 1.0115x over previous
"""Multi-head attention (N=2, L=2048, D=1024, H=16) on 8 NeuronCores.

Sharding: core c -> (batch n = c // 4, head group g = c % 4, 4 heads each).
Each core computes Q/K/V projections for its 4 heads, attention, and its
slice of the output projection. Host sums the 4 partial output projections
per batch and adds bo.

v2 design (from baseline profiling: PE 210.7us busy, exec 260us, 31.5us
PE gaps + ~25us HAM cold-clock tax + 12.4us tail):
- Host-blocked qt-major x layouts ([QT, 128, DC*512], 8KB DMA lines) so
  inputs arrive in consumption order; kproj(0)/vproj(0)/qproj(0) feed at
  ~13-19us and the attention pipeline ignites at ~22us.
- Single-head rounds: per (qb, hp, sk, hh): S^T = 2 MMs into one
  [128,1024] fp32 psum tile (ring 2), one exp (ScalarE, scale=1/32),
  PV = 2 MMs (M=65: V dims + ones column accumulating the softmax
  denominator). Ring granularity == round granularity so the exp stream
  never stalls on psum (baseline allocated 4 tiles/round vs ring 2).
- Head-pair outer loop (hp): only 2 PV accumulators live -> PSUM fits:
  s(2x2 banks) + acc(2x1) + o(2x1) = 8 banks, leaving an "o" ring for
  interleaved filler matmuls (projections + prev-qb out-proj).
- Out-proj packs head pairs: oN2[hp] [128,512] holds both heads' dims ->
  K=128 fully used, 8 MMs/qb instead of 16. Odd head's normalized rows
  are written at partition offset 64 (SHIFT_MODE selects DVE direct
  offset write vs PE identity-shift matmul).
- Reciprocal on DVE: bc matmul broadcasts the RAW denominator row (K=1
  matmul from partition 64), nc.vector.reciprocal on [64,512], then one
  fused tensor_tensor multiply -> ScalarE runs exps only (no Ln/Exp
  pair, no act-table patching).
- PE warmup matmuls on memset tiles during the DMA lead-in (HAM).
"""
import os
import sys
import types

import numpy as np

N_BATCH = 2
L = 2048
D = 1024
H = 16
HD = 64
CORES = 8
GH = 4            # heads per core
DG = GH * HD      # 256 = projected dims per core
QB = 512          # q block
KT = L // 128     # 16 k tiles
QT = L // QB      # 4 q blocks
DC = D // 128     # 8 din chunks
SCALE = 1.0 / 32.0  # 1/sqrt(D)
SHIFT_MODE = "dve"  # "dve": direct offset write; "pe": identity matmul shift


def _install_ntff_hook():
    """The image's antenv stub lacks axon_hooks; shim it so trace=True works."""
    if "antenv.axon_hooks" in sys.modules:
        return
    mod = types.ModuleType("antenv.axon_hooks")
    mod._hook = None
    mod.set_axon_ntff_profile_hook = lambda h: setattr(mod, "_hook", h)
    mod.get_axon_ntff_profile_hook = lambda: mod._hook
    sys.modules["antenv.axon_hooks"] = mod
    try:
        from trn_agent_boot.trn_boot import _ntff_profile_via_ctypes
        mod._hook = _ntff_profile_via_ctypes("/opt/axon/libaxon_pjrt.so")
    except Exception:
        mod._hook = None


_install_ntff_hook()

import concourse.bacc as bacc
import concourse.mybir as mybir
import concourse.tile as tile
from concourse.bass_utils import run_bass_kernel_spmd

F32 = mybir.dt.float32
F16 = mybir.dt.float16
AF = mybir.ActivationFunctionType
MULT = mybir.AluOpType.mult

_CACHE = {}


def _build(use_bias, use_mask):
    key = (use_bias, use_mask)
    if key in _CACHE:
        return _CACHE[key]

    nc = bacc.Bacc("TRN2", debug=False, num_devices=CORES)

    xqb = nc.dram_tensor("xqb", [QT * 128, DC * 512], F16, kind="ExternalInput").ap()
    xkb = nc.dram_tensor("xkb", [QT * 128, DC * 512], F16, kind="ExternalInput").ap()
    xvb = nc.dram_tensor("xvb", [QT * 128, DC * 512], F16, kind="ExternalInput").ap()
    aq = nc.dram_tensor("aq", [128, DC * DG], F16, kind="ExternalInput").ap()
    ak = nc.dram_tensor("ak", [128, DC * DG], F16, kind="ExternalInput").ap()
    av = nc.dram_tensor("av", [128, DC * DG], F16, kind="ExternalInput").ap()
    bo = nc.dram_tensor("bo", [128, 2 * D], F16, kind="ExternalInput").ap()
    bq = nc.dram_tensor("bq", [1, DG], F16, kind="ExternalInput").ap()
    bk = nc.dram_tensor("bk", [1, DG], F16, kind="ExternalInput").ap()
    bv = nc.dram_tensor("bv", [1, DG], F16, kind="ExternalInput").ap()
    eye = nc.dram_tensor("eye", [64, 64], F16, kind="ExternalInput").ap()
    maskf = nc.dram_tensor("maskf", [128, KT], F32, kind="ExternalInput").ap()
    outp = nc.dram_tensor("outp", [L, D], F16, kind="ExternalOutput").ap()

    with tile.TileContext(nc) as tc:
        _emit(nc, tc, dict(xqb=xqb, xkb=xkb, xvb=xvb, aq=aq, ak=ak, av=av,
                           bo=bo, bq=bq, bk=bk, bv=bv, eye=eye, maskf=maskf,
                           outp=outp),
              use_bias, use_mask)
    nc.compile()
    _CACHE[key] = nc
    return nc


def _emit(nc, tc, t, use_bias, use_mask):
    from contextlib import ExitStack
    ctx = ExitStack()
    with ctx:
        sb_w = ctx.enter_context(tc.tile_pool(name="sb_w", bufs=1))
        sb_qkv = ctx.enter_context(tc.tile_pool(name="sb_qkv", bufs=1))
        sb_pt = ctx.enter_context(tc.tile_pool(name="sb_pt", bufs=4))
        sb_n = ctx.enter_context(tc.tile_pool(name="sb_n", bufs=4))
        sb_out = ctx.enter_context(tc.tile_pool(name="sb_out", bufs=3))
        ps = ctx.enter_context(tc.tile_pool(name="ps", bufs=2, space="PSUM"))

        # ---- resident tiles ----
        aq_t = sb_w.tile([128, DC, DG], F16, tag="aq")
        ak_t = sb_w.tile([128, DC, DG], F16, tag="ak")
        av_t = sb_w.tile([128, DC, DG], F16, tag="av")
        bo_t = sb_w.tile([128, 2, D], F16, tag="bo")
        ones_t = sb_w.tile([128, 512], F16, tag="ones")
        eye_t = sb_w.tile([64, 64], F16, tag="eye")
        xq_res = sb_w.tile([128, QT, DC * 512], F16, tag="xq")
        xk_res = sb_w.tile([128, QT, DC * 512], F16, tag="xk")
        xv_res = sb_w.tile([128, QT, DC * 512], F16, tag="xv")
        KT_sb = [sb_qkv.tile([128, L], F16, tag=f"kt{m}", name=f"KTm{m}")
                 for m in range(2)]
        QT_z = [sb_qkv.tile([128, L], F16, tag=f"qz{h}", name=f"QTz{h}")
                for h in range(GH)]
        V1 = sb_qkv.tile([128, KT, GH, HD + 1], F16, tag="v1")
        # oN2[qb%2][hp]: packed normalized heads for the out-projection
        oN2 = [[sb_qkv.tile([128, 512], F16, tag=f"oN{b}{hp}",
                            name=f"oN{b}{hp}") for hp in range(2)]
               for b in range(2)]

        # ---- warmup tiles (no DMA deps; HAM ramp during input stream) ----
        warm_w = sb_w.tile([128, 128], F16, tag="warmw")
        warm_x = sb_w.tile([128, 512], F16, tag="warmx")
        nc.vector.memset(warm_w, 0.0)
        nc.vector.memset(warm_x, 0.0)
        nc.vector.memset(ones_t, 1.0)
        # ACT table warmup (exp)
        warm_a = sb_w.tile([1, 32], F32, tag="warma")
        nc.vector.memset(warm_a, 1.0)
        warm_b = sb_w.tile([1, 32], F32, tag="warmb")
        nc.scalar.activation(out=warm_b, in_=warm_a, func=AF.Exp)

        for h in range(GH):
            z0 = 0 if h % 2 else 64
            nc.vector.memset(QT_z[h][z0:z0 + 64, :], 0.0)

        # ---- input DMAs: one priority-ordered queue (sync) ----
        # First tiles split in halves so qproj/kproj start ~2us earlier.
        def dma_x(res, src, qt, half=None):
            if half is None:
                nc.sync.dma_start(out=res[:, qt, :],
                                  in_=src[qt * 128:(qt + 1) * 128, :])
            else:
                h0 = half * (DC // 2) * 512
                h1 = h0 + (DC // 2) * 512
                nc.sync.dma_start(out=res[:, qt, h0:h1],
                                  in_=src[qt * 128:(qt + 1) * 128, h0:h1])

        nc.sync.dma_start(out=aq_t, in_=t["aq"].rearrange("p (c d) -> p c d", c=DC))
        dma_x(xq_res, t["xqb"], 0, 0)
        dma_x(xq_res, t["xqb"], 0, 1)
        nc.sync.dma_start(out=ak_t, in_=t["ak"].rearrange("p (c d) -> p c d", c=DC))
        dma_x(xk_res, t["xkb"], 0, 0)
        dma_x(xk_res, t["xkb"], 0, 1)
        nc.sync.dma_start(out=av_t, in_=t["av"].rearrange("p (c d) -> p c d", c=DC))
        dma_x(xv_res, t["xvb"], 0, 0)
        dma_x(xv_res, t["xvb"], 0, 1)
        for qt in range(1, QT):
            dma_x(xk_res, t["xkb"], qt)
            dma_x(xv_res, t["xvb"], qt)
            dma_x(xq_res, t["xqb"], qt)
        nc.sync.dma_start(out=bo_t, in_=t["bo"].rearrange("p (a d) -> p a d", a=2))
        if SHIFT_MODE == "pe":
            nc.sync.dma_start(out=eye_t, in_=t["eye"])
        if use_mask:
            mask_t = sb_w.tile([128, KT], F32, tag="mask")
            nc.sync.dma_start(out=mask_t, in_=t["maskf"])
        bq_t = bk_t = bv_t = None
        if use_bias:
            bq_t = sb_w.tile([1, DG], F16, tag="bq")
            bk_t = sb_w.tile([1, DG], F16, tag="bk")
            bv_t = sb_w.tile([1, DG], F16, tag="bv")
            nc.sync.dma_start(out=bq_t, in_=t["bq"])
            nc.sync.dma_start(out=bk_t, in_=t["bk"])
            nc.sync.dma_start(out=bv_t, in_=t["bv"])

        # V1 ones column (column HD of every (kt, h) slot)
        if use_mask:
            ones4 = sb_w.tile([128, GH], F32, tag="ones4")
            nc.vector.memset(ones4, 1.0)
            for kt in range(KT):
                nc.vector.tensor_scalar_mul(
                    V1[:, kt, :, HD:HD + 1],
                    ones4.rearrange("p h -> p h 1"), mask_t[:, kt:kt + 1])
        else:
            nc.vector.memset(V1[:, :, :, HD:HD + 1], 1.0)

        # ---- PE warmup: dummy matmuls to ramp HAM while inputs stream ----
        for w in range(14):
            psw = ps.tile([128, 512], F32, tag="o", bufs=2, name=f"psw_{w}")
            nc.tensor.matmul(psw[:, 0:512], warm_w, warm_x,
                             start=True, stop=True)

        # ---- emit helpers ----
        def emit_qproj(qb, p):
            # packed head pair p: one M=128 matmul per c chunk
            psq = ps.tile([128, 512], F32, tag="o", bufs=2, name=f"psq_{qb}_{p}")
            for c in range(DC):
                xsl = xq_res[:, qb, c * 512:(c + 1) * 512]
                nc.tensor.matmul(
                    psq[:, 0:512], aq_t[:, c, p * 128:(p + 1) * 128], xsl,
                    start=(c == 0), stop=(c == DC - 1 and not use_bias))
            if use_bias:
                nc.tensor.matmul(
                    psq[:, 0:512], bq_t[:, p * 128:(p + 1) * 128],
                    ones_t[0:1, :], start=False, stop=True)
            for hh in range(2):
                h = p * 2 + hh
                r0 = 64 * hh
                nc.vector.tensor_copy(
                    QT_z[h][r0:r0 + 64, qb * 512:(qb + 1) * 512],
                    psq[r0:r0 + 64, 0:512])

        def emit_kproj(qt, m):
            psm = ps.tile([128, 512], F32, tag="o", bufs=2, name=f"psk_{qt}_{m}")
            for c in range(DC):
                xsl = xk_res[:, qt, c * 512:(c + 1) * 512]
                nc.tensor.matmul(
                    psm[:, 0:512], ak_t[:, c, m * 128:(m + 1) * 128], xsl,
                    start=(c == 0), stop=(c == DC - 1 and not use_bias))
            if use_bias:
                nc.tensor.matmul(
                    psm[:, 0:512], bk_t[:, m * 128:(m + 1) * 128],
                    ones_t[0:1, :], start=False, stop=True)
            nc.vector.tensor_copy(
                KT_sb[m][:, qt * 512:(qt + 1) * 512], psm[:, 0:512])

        def emit_vproj(ktg, j):
            psv = ps.tile([128, 512], F32, tag="o", bufs=2, name=f"psv_{ktg}_{j}")
            for c in range(DC):
                xsl = xv_res[:, ktg, c * 512:(c + 1) * 512]
                nc.tensor.matmul(
                    psv[:, 0:DG], xsl[:, j * 128:(j + 1) * 128],
                    av_t[:, c, :],
                    start=(c == 0), stop=(c == DC - 1 and not use_bias))
            if use_bias:
                nc.tensor.matmul(
                    psv[:, 0:DG], ones_t[0:1, 0:128], bv_t,
                    start=False, stop=True)
            kt = ktg * 4 + j
            srcv = psv[:, 0:DG].rearrange("p (h d) -> p h d", h=GH)
            if use_mask:
                nc.vector.tensor_scalar_mul(
                    V1[:, kt, :, 0:HD], srcv, mask_t[:, kt:kt + 1])
            else:
                nc.vector.tensor_copy(V1[:, kt, :, 0:HD], srcv)

        # ---- the attention round engine ----
        # round r = (qb, hp, sk, hh): S^T (2 MMs -> pss), exp, then PV of
        # the PREVIOUS round (software pipeline, 1-round lag).
        filler = []           # list of closures, each ~2 matmuls
        fill_debt = [0.0]     # fractional chunks owed

        def pop_filler(n=1.0):
            fill_debt[0] += n
            while fill_debt[0] >= 1.0 and filler:
                filler.pop(0)()
                fill_debt[0] -= 1.0

        def emit_st(qb, hp, sk, hh):
            h = hp * 2 + hh
            pss = ps.tile([128, 1024], F32, tag="s", bufs=2,
                          name=f"pss_{qb}_{sk}_{h}")
            for dk in range(2):
                kt = sk * 2 + dk
                nc.tensor.matmul(
                    pss[:, dk * 512:(dk + 1) * 512],
                    KT_sb[hp][:, kt * 128:(kt + 1) * 128],
                    QT_z[h][:, qb * 512:qb * 512 + QB],
                    start=True, stop=True)
            pt = sb_pt.tile([128, 1024], F16, tag="pt", bufs=6,
                            name=f"pt_{qb}_{sk}_{h}")
            nc.scalar.activation(out=pt, in_=pss, func=AF.Exp, scale=SCALE)
            return pt

        def emit_pv(qb, hp, sk, hh, pt, pso):
            h = hp * 2 + hh
            for dk in range(2):
                kt = sk * 2 + dk
                nc.tensor.matmul(
                    pso[hh][0:HD + 1, :], V1[:, kt, h, :],
                    pt[:, dk * 512:(dk + 1) * 512],
                    start=(kt == 0), stop=(kt == KT - 1))

        def emit_hp_tail(qb, hp, pso):
            # normalize both heads of the pair into oN2[qb%2][hp];
            # the two heads' chains are interleaved so engines pipeline
            on = oN2[qb % 2][hp]
            oTs, bcs, rcps = [], [], []
            for hh in range(2):
                oT = sb_n.tile([HD + 1, 512], F16, tag="oT", bufs=4,
                               name=f"oT_{qb}_{hp}_{hh}")
                nc.vector.tensor_copy(oT, pso[hh][0:HD + 1, :])
                oTs.append(oT)
            for hh in range(2):
                bc = ps.tile([128, 512], F32, tag="o", bufs=2,
                             name=f"bc_{qb}_{hp}_{hh}")
                nc.tensor.matmul(
                    bc[0:64, :], ones_t[64:65, 0:64], oTs[hh][64:65, :],
                    start=True, stop=True, tile_position=(64, 0))
                bcs.append(bc)
            for hh in range(2):
                rcp = sb_n.tile([64, 512], F32, tag="rcp", bufs=2,
                                name=f"rcp_{qb}_{hp}_{hh}")
                nc.vector.reciprocal_approx_fast(out=rcp, in_=bcs[hh][0:64, :])
                rcps.append(rcp)
            for hh in range(2):
                rows = slice(0, 64) if hh == 0 else slice(64, 128)
                nc.vector.tensor_tensor(on[rows, :], oTs[hh][0:64, :],
                                        rcps[hh], op=MULT)

        def emit_outproj_chunk(qb, mq, nb, psout_box, tag="o"):
            # two packed MMs (hp 0,1) accumulating psout, then CAST out
            on_pair = oN2[qb % 2]
            psout = ps.tile([128, 512], F32, tag=tag, bufs=2,
                            name=f"psout_{qb}_{mq}_{nb}")
            for hp in range(2):
                nc.tensor.matmul(
                    psout[:, 0:512],
                    on_pair[hp][:, mq * 128:(mq + 1) * 128],
                    bo_t[:, hp, nb * 512:(nb + 1) * 512],
                    start=(hp == 0), stop=(hp == 1))
            psout_box[nb] = psout

        def emit_outproj(qb, deep=False):
            # returns filler closures: 8 chunks of 2 MMs + CAST/DMA.
            # deep=True (endgame): alternate psum tags "o"/"s" so 4 psout
            # tiles pipeline (the attention rings are drained by then).
            chunks = []
            for mq in range(4):
                ot = sb_out.tile([128, D], F16, tag="ot", name=f"ot_{qb}_{mq}")
                box = {}
                tg0 = "s" if deep and mq % 2 else "o"
                tg1 = "s" if deep and not mq % 2 else "o"

                def mk(qb=qb, mq=mq, ot=ot, box=box, tg0=tg0, tg1=tg1):
                    def c0():
                        emit_outproj_chunk(qb, mq, 0, box, tg0)
                        nc.vector.tensor_copy(ot[:, 0:512], box[0][:, 0:512])

                    def c1():
                        emit_outproj_chunk(qb, mq, 1, box, tg1)
                        nc.vector.tensor_copy(ot[:, 512:1024], box[1][:, 0:512])
                        q0 = qb * QB + mq * 128
                        nc.gpsimd.dma_start(out=t["outp"][q0:q0 + 128, :],
                                            in_=ot)
                    return [c0, c1]
                chunks.extend(mk())
            return chunks

        # ---- schedule ----
        emit_qproj(0, 0)
        emit_qproj(0, 1)
        emit_kproj(0, 0)
        emit_kproj(0, 1)
        for j in range(4):
            emit_vproj(0, j)

        # qb0 filler maps (round index -> closures), deadline-correct:
        # kproj(qt,0) must be EMITTED before hp0's S^T of sk=2qt (round
        # 4qt... ridx 2sk+hh); vproj(qt,j) before the PV of the sk
        # covering kt (PV lags one round); kproj(*,1) before hp1's rounds.
        sched_q0h0 = {
            0: [lambda: emit_kproj(1, 0)],
            1: [lambda: emit_vproj(1, 0)],
            2: [lambda: emit_vproj(1, 1)],
            3: [lambda: emit_vproj(1, 2), lambda: emit_vproj(1, 3)],
            4: [lambda: emit_kproj(2, 0)],
            5: [lambda: emit_vproj(2, 0)],
            6: [lambda: emit_vproj(2, 1)],
            7: [lambda: emit_vproj(2, 2), lambda: emit_vproj(2, 3)],
            8: [lambda: emit_kproj(3, 0)],
            9: [lambda: emit_vproj(3, 0)],
            10: [lambda: emit_vproj(3, 1)],
            11: [lambda: emit_vproj(3, 2), lambda: emit_vproj(3, 3)],
            12: [lambda: emit_kproj(1, 1)],
            13: [lambda: emit_kproj(2, 1)],
            14: [lambda: emit_kproj(3, 1)],
        }
        sched_q0h1 = {
            0: [lambda: emit_qproj(1, 1)],
            6: [lambda: emit_qproj(2, 0)],
        }

        def spread(items, n_rounds=32, reserve=2):
            """Assign items evenly to round indices, keeping `reserve`
            items back for the hp-tail boundaries."""
            body = items[:len(items) - reserve] if reserve else items
            tail = items[len(items) - reserve:] if reserve else []
            m = {}
            if body:
                for i, it in enumerate(body):
                    m.setdefault(i * n_rounds // len(body), []).append(it)
            return m, tail

        def run_qb(qb, s0, s1, t0, t1):
            for hp in range(2):
                sched = s0 if hp == 0 else s1
                tailf = t0 if hp == 0 else t1
                pso = [ps.tile([128, 512], F32, tag="acc", bufs=2,
                               name=f"pso_{qb}_{hp}_{hh}") for hh in range(2)]
                prev = None
                ridx = 0
                for sk in range(8):
                    for hh in range(2):
                        # order: S^T(r), filler, PV(r-1) — the filler MMs
                        # stream while exp(r-1) finishes, so the PV lands
                        # back-to-back instead of draining isolated
                        pt = emit_st(qb, hp, sk, hh)
                        for fn in sched.pop(ridx, []):
                            fn()
                        if prev is not None:
                            emit_pv(*prev)
                        prev = (qb, hp, sk, hh, pt, pso)
                        ridx += 1
                emit_pv(*prev)
                for fn in tailf:
                    fn()
                emit_hp_tail(qb, hp, pso)

        run_qb(0, sched_q0h0, sched_q0h1,
               [lambda: emit_qproj(1, 0)], [lambda: emit_qproj(2, 1)])
        for qb in range(1, QT):
            items = emit_outproj(qb - 1)
            if qb == 2:
                items += [lambda: emit_qproj(3, 0), lambda: emit_qproj(3, 1)]
            half0, rest = items[:len(items) // 2], items[len(items) // 2:]
            s0, t0 = spread(half0, n_rounds=16, reserve=1)
            s1, t1 = spread(rest, n_rounds=16, reserve=2 if qb == QT - 1 else 1)
            run_qb(qb, s0, s1, t0, t1)
        for ch in emit_outproj(QT - 1, deep=True):
            ch()


def _swizzle_a(aT):
    """[D, DG] -> [128, DC*DG]: partition p holds chunks c at (c, :)."""
    return np.ascontiguousarray(
        aT.reshape(DC, 128, DG).transpose(1, 0, 2).reshape(128, DC * DG))


def _pack_bo(boT):
    """[DG, D] -> [128, 2*D]: head-pair hp at cols hp*D, rows=pair dims."""
    out = boT.reshape(2, 128, D).transpose(1, 0, 2)
    return np.ascontiguousarray(out.reshape(128, 2 * D))


def _block_x(xT):
    """[D, L] -> [QT*128, DC*512] qt-major blocks, 8KB partition lines."""
    return np.ascontiguousarray(
        xT.reshape(DC, 128, QT, 512).transpose(2, 1, 0, 3).reshape(
            QT * 128, DC * 512))


_EYE = np.eye(64, dtype=np.float16)


def _prep_inputs(values, key, query, mask, Wv, Wk, Wq, Wo, bv, bk, bq):
    """Build the 8 per-core input maps (host-side shard + layout)."""
    xB = {}
    for n in range(N_BATCH):
        xB[("q", n)] = _block_x(query[n].T.astype(np.float16))
        xB[("k", n)] = _block_x(key[n].T.astype(np.float16))
        xB[("v", n)] = _block_x(values[n].T.astype(np.float16))
    in_maps = []
    for c in range(CORES):
        n, g = divmod(c, CORES // N_BATCH)
        rows = slice(g * DG, (g + 1) * DG)
        mrow = np.ascontiguousarray(
            mask[n, 0, 0, :].astype(np.float32).reshape(KT, 128).T)
        in_maps.append({
            "xqb": xB[("q", n)],
            "xkb": xB[("k", n)],
            "xvb": xB[("v", n)],
            "aq": _swizzle_a(Wq[rows, :].T.astype(np.float16)),
            "ak": _swizzle_a(Wk[rows, :].T.astype(np.float16)),
            "av": _swizzle_a(Wv[rows, :].T.astype(np.float16)),
            "bo": _pack_bo(Wo[:, rows].T.astype(np.float16)),
            "bq": np.ascontiguousarray(bq[None, rows].astype(np.float16)),
            "bk": np.ascontiguousarray(bk[None, rows].astype(np.float16)),
            "bv": np.ascontiguousarray(bv[None, rows].astype(np.float16)),
            "eye": _EYE,
            "maskf": mrow,
        })
    return in_maps


LAST_EXEC_NS = None
LAST_RES = None


def kernel(values, key, query, mask, Wv, bv, Wk, bk, Wq, bq, Wo, bo,
           trace=False, trace_cores=None):
    global LAST_EXEC_NS, LAST_RES
    values = np.asarray(values, dtype=np.float32)
    key = np.asarray(key, dtype=np.float32)
    query = np.asarray(query, dtype=np.float32)
    mask = np.asarray(mask)
    Wq, Wk, Wv, Wo = (np.asarray(Wq, np.float32), np.asarray(Wk, np.float32),
                      np.asarray(Wv, np.float32), np.asarray(Wo, np.float32))
    bq, bk, bv, bo = (np.asarray(bq, np.float32), np.asarray(bk, np.float32),
                      np.asarray(bv, np.float32), np.asarray(bo, np.float32))

    use_bias = bool(np.any(bq) or np.any(bk) or np.any(bv))
    use_mask = not bool(np.all(np.asarray(mask) == 1))

    nc = _build(use_bias, use_mask)
    in_maps = _prep_inputs(values, key, query, mask, Wv, Wk, Wq, Wo,
                           bv, bk, bq)
    kw = {}
    if trace_cores is not None:
        kw["trace_cores"] = trace_cores
    res = run_bass_kernel_spmd(nc, in_maps, core_ids=list(range(CORES)),
                               trace=trace, **kw)
    LAST_EXEC_NS = res.exec_time_ns
    LAST_RES = res

    out = np.zeros((N_BATCH, L, D), dtype=np.float32)
    for c in range(CORES):
        n = c // (CORES // N_BATCH)
        out[n] += res.results[c]["outp"].astype(np.float32)
    out += bo[None, None, :]
    return out


# revision 17
# speedup vs baseline: 1.2360x; 1.0372x over previous
"""Multi-head attention (N=2, L=2048, D=1024, H=16) on 8 NeuronCores.

Sharding: core c -> (batch n = c // 4, head group g = c % 4, 4 heads each).
Each core computes Q/K/V projections for its 4 heads, attention, and its
slice of the output projection. Host sums the 4 partial output projections
per batch and adds bo.

v2 design (from baseline profiling: PE 210.7us busy, exec 260us, 31.5us
PE gaps + ~25us HAM cold-clock tax + 12.4us tail):
- Host-blocked qt-major x layouts ([QT, 128, DC*512], 8KB DMA lines) so
  inputs arrive in consumption order; kproj(0)/vproj(0)/qproj(0) feed at
  ~13-19us and the attention pipeline ignites at ~22us.
- Single-head rounds: per (qb, hp, sk, hh): S^T = 2 MMs into one
  [128,1024] fp32 psum tile (ring 2), one exp (ScalarE, scale=1/32),
  PV = 2 MMs (M=65: V dims + ones column accumulating the softmax
  denominator). Ring granularity == round granularity so the exp stream
  never stalls on psum (baseline allocated 4 tiles/round vs ring 2).
- Head-pair outer loop (hp): only 2 PV accumulators live -> PSUM fits:
  s(2x2 banks) + acc(2x1) + o(2x1) = 8 banks, leaving an "o" ring for
  interleaved filler matmuls (projections + prev-qb out-proj).
- Out-proj packs head pairs: oN2[hp] [128,512] holds both heads' dims ->
  K=128 fully used, 8 MMs/qb instead of 16. Odd head's normalized rows
  are written at partition offset 64 (SHIFT_MODE selects DVE direct
  offset write vs PE identity-shift matmul).
- Reciprocal on DVE: bc matmul broadcasts the RAW denominator row (K=1
  matmul from partition 64), nc.vector.reciprocal on [64,512], then one
  fused tensor_tensor multiply -> ScalarE runs exps only (no Ln/Exp
  pair, no act-table patching).
- PE warmup matmuls on memset tiles during the DMA lead-in (HAM).
"""
import os
import sys
import types

import numpy as np

N_BATCH = 2
L = 2048
D = 1024
H = 16
HD = 64
CORES = 8
GH = 4            # heads per core
DG = GH * HD      # 256 = projected dims per core
QB = 512          # q block
KT = L // 128     # 16 k tiles
QT = L // QB      # 4 q blocks
DC = D // 128     # 8 din chunks
SCALE = 1.0 / 32.0  # 1/sqrt(D)
SHIFT_MODE = "dve"  # "dve": direct offset write; "pe": identity matmul shift


def _install_ntff_hook():
    """The image's antenv stub lacks axon_hooks; shim it so trace=True works."""
    if "antenv.axon_hooks" in sys.modules:
        return
    mod = types.ModuleType("antenv.axon_hooks")
    mod._hook = None
    mod.set_axon_ntff_profile_hook = lambda h: setattr(mod, "_hook", h)
    mod.get_axon_ntff_profile_hook = lambda: mod._hook
    sys.modules["antenv.axon_hooks"] = mod
    try:
        from trn_agent_boot.trn_boot import _ntff_profile_via_ctypes
        mod._hook = _ntff_profile_via_ctypes("/opt/axon/libaxon_pjrt.so")
    except Exception:
        mod._hook = None


_install_ntff_hook()

import concourse.bacc as bacc
import concourse.mybir as mybir
import concourse.tile as tile
from concourse.bass_utils import run_bass_kernel_spmd

F32 = mybir.dt.float32
F16 = mybir.dt.float16
AF = mybir.ActivationFunctionType
MULT = mybir.AluOpType.mult

_CACHE = {}


def _build(use_bias, use_mask):
    key = (use_bias, use_mask)
    if key in _CACHE:
        return _CACHE[key]

    nc = bacc.Bacc("TRN2", debug=False, num_devices=CORES)

    xqb = nc.dram_tensor("xqb", [QT * 128, DC * 512], F16, kind="ExternalInput").ap()
    xkb = nc.dram_tensor("xkb", [QT * 128, DC * 512], F16, kind="ExternalInput").ap()
    xvb = nc.dram_tensor("xvb", [QT * 128, DC * 512], F16, kind="ExternalInput").ap()
    aq = nc.dram_tensor("aq", [128, DC * DG], F16, kind="ExternalInput").ap()
    ak = nc.dram_tensor("ak", [128, DC * DG], F16, kind="ExternalInput").ap()
    av = nc.dram_tensor("av", [128, DC * DG], F16, kind="ExternalInput").ap()
    bo = nc.dram_tensor("bo", [128, 2 * D], F16, kind="ExternalInput").ap()
    bq = nc.dram_tensor("bq", [1, DG], F16, kind="ExternalInput").ap()
    bk = nc.dram_tensor("bk", [1, DG], F16, kind="ExternalInput").ap()
    bv = nc.dram_tensor("bv", [1, DG], F16, kind="ExternalInput").ap()
    eye = nc.dram_tensor("eye", [64, 64], F16, kind="ExternalInput").ap()
    maskf = nc.dram_tensor("maskf", [128, KT], F32, kind="ExternalInput").ap()
    outp = nc.dram_tensor("outp", [L, D], F16, kind="ExternalOutput").ap()

    with tile.TileContext(nc) as tc:
        _emit(nc, tc, dict(xqb=xqb, xkb=xkb, xvb=xvb, aq=aq, ak=ak, av=av,
                           bo=bo, bq=bq, bk=bk, bv=bv, eye=eye, maskf=maskf,
                           outp=outp),
              use_bias, use_mask)
    nc.compile()
    _CACHE[key] = nc
    return nc


def _emit(nc, tc, t, use_bias, use_mask):
    from contextlib import ExitStack
    ctx = ExitStack()
    with ctx:
        sb_w = ctx.enter_context(tc.tile_pool(name="sb_w", bufs=1))
        sb_qkv = ctx.enter_context(tc.tile_pool(name="sb_qkv", bufs=1))
        sb_pt = ctx.enter_context(tc.tile_pool(name="sb_pt", bufs=4))
        sb_n = ctx.enter_context(tc.tile_pool(name="sb_n", bufs=4))
        sb_out = ctx.enter_context(tc.tile_pool(name="sb_out", bufs=3))
        ps = ctx.enter_context(tc.tile_pool(name="ps", bufs=2, space="PSUM"))

        # ---- resident tiles ----
        aq_t = sb_w.tile([128, DC, DG], F16, tag="aq")
        ak_t = sb_w.tile([128, DC, DG], F16, tag="ak")
        av_t = sb_w.tile([128, DC, DG], F16, tag="av")
        bo_t = sb_w.tile([128, 2, D], F16, tag="bo")
        ones_t = sb_w.tile([128, 512], F16, tag="ones")
        eye_t = sb_w.tile([64, 64], F16, tag="eye")
        xq_res = sb_w.tile([128, QT, DC * 512], F16, tag="xq")
        xk_res = sb_w.tile([128, QT, DC * 512], F16, tag="xk")
        xv_res = sb_w.tile([128, QT, DC * 512], F16, tag="xv")
        KT_sb = [sb_qkv.tile([128, L], F16, tag=f"kt{m}", name=f"KTm{m}")
                 for m in range(2)]
        QT_z = [sb_qkv.tile([128, L], F16, tag=f"qz{h}", name=f"QTz{h}")
                for h in range(GH)]
        V1 = sb_qkv.tile([128, KT, GH, HD + 1], F16, tag="v1")
        # oN2[qb%2][hp]: packed normalized heads for the out-projection
        oN2 = [[sb_qkv.tile([128, 512], F16, tag=f"oN{b}{hp}",
                            name=f"oN{b}{hp}") for hp in range(2)]
               for b in range(2)]

        # ---- warmup tiles (no DMA deps; HAM ramp during input stream) ----
        warm_w = sb_w.tile([128, 128], F16, tag="warmw")
        warm_x = sb_w.tile([128, 512], F16, tag="warmx")
        nc.vector.memset(warm_w, 0.0)
        nc.vector.memset(warm_x, 0.0)
        nc.vector.memset(ones_t, 1.0)
        # ACT table warmup (exp)
        warm_a = sb_w.tile([1, 32], F32, tag="warma")
        nc.vector.memset(warm_a, 1.0)
        warm_b = sb_w.tile([1, 32], F32, tag="warmb")
        nc.scalar.activation(out=warm_b, in_=warm_a, func=AF.Exp)

        for h in range(GH):
            z0 = 0 if h % 2 else 64
            nc.vector.memset(QT_z[h][z0:z0 + 64, :], 0.0)

        # ---- input DMAs: one priority-ordered queue (sync) ----
        # First tiles split in halves so qproj/kproj start ~2us earlier.
        def dma_x(res, src, qt, half=None):
            if half is None:
                nc.sync.dma_start(out=res[:, qt, :],
                                  in_=src[qt * 128:(qt + 1) * 128, :])
            else:
                h0 = half * (DC // 2) * 512
                h1 = h0 + (DC // 2) * 512
                nc.sync.dma_start(out=res[:, qt, h0:h1],
                                  in_=src[qt * 128:(qt + 1) * 128, h0:h1])

        nc.sync.dma_start(out=aq_t, in_=t["aq"].rearrange("p (c d) -> p c d", c=DC))
        dma_x(xq_res, t["xqb"], 0, 0)
        dma_x(xq_res, t["xqb"], 0, 1)
        nc.sync.dma_start(out=ak_t, in_=t["ak"].rearrange("p (c d) -> p c d", c=DC))
        dma_x(xk_res, t["xkb"], 0, 0)
        dma_x(xk_res, t["xkb"], 0, 1)
        nc.sync.dma_start(out=av_t, in_=t["av"].rearrange("p (c d) -> p c d", c=DC))
        dma_x(xv_res, t["xvb"], 0, 0)
        dma_x(xv_res, t["xvb"], 0, 1)
        for qt in range(1, QT):
            dma_x(xk_res, t["xkb"], qt)
            dma_x(xv_res, t["xvb"], qt)
            dma_x(xq_res, t["xqb"], qt)
        nc.sync.dma_start(out=bo_t, in_=t["bo"].rearrange("p (a d) -> p a d", a=2))
        if SHIFT_MODE == "pe":
            nc.sync.dma_start(out=eye_t, in_=t["eye"])
        if use_mask:
            mask_t = sb_w.tile([128, KT], F32, tag="mask")
            nc.sync.dma_start(out=mask_t, in_=t["maskf"])
        bq_t = bk_t = bv_t = None
        if use_bias:
            bq_t = sb_w.tile([1, DG], F16, tag="bq")
            bk_t = sb_w.tile([1, DG], F16, tag="bk")
            bv_t = sb_w.tile([1, DG], F16, tag="bv")
            nc.sync.dma_start(out=bq_t, in_=t["bq"])
            nc.sync.dma_start(out=bk_t, in_=t["bk"])
            nc.sync.dma_start(out=bv_t, in_=t["bv"])

        # V1 ones column (column HD of every (kt, h) slot)
        if use_mask:
            ones4 = sb_w.tile([128, GH], F32, tag="ones4")
            nc.vector.memset(ones4, 1.0)
            for kt in range(KT):
                nc.vector.tensor_scalar_mul(
                    V1[:, kt, :, HD:HD + 1],
                    ones4.rearrange("p h -> p h 1"), mask_t[:, kt:kt + 1])
        else:
            nc.vector.memset(V1[:, :, :, HD:HD + 1], 1.0)

        # ---- PE warmup: dummy matmuls to ramp HAM while inputs stream ----
        for w in range(10):
            psw = ps.tile([128, 512], F32, tag="o", bufs=2, name=f"psw_{w}")
            nc.tensor.matmul(psw[:, 0:512], warm_w, warm_x,
                             start=True, stop=True)

        # ---- emit helpers ----
        def emit_qproj(qb, p):
            # packed head pair p: one M=128 matmul per c chunk
            psq = ps.tile([128, 512], F32, tag="o", bufs=2, name=f"psq_{qb}_{p}")
            for c in range(DC):
                xsl = xq_res[:, qb, c * 512:(c + 1) * 512]
                nc.tensor.matmul(
                    psq[:, 0:512], aq_t[:, c, p * 128:(p + 1) * 128], xsl,
                    start=(c == 0), stop=(c == DC - 1 and not use_bias))
            if use_bias:
                nc.tensor.matmul(
                    psq[:, 0:512], bq_t[:, p * 128:(p + 1) * 128],
                    ones_t[0:1, :], start=False, stop=True)
            for hh in range(2):
                h = p * 2 + hh
                r0 = 64 * hh
                nc.vector.tensor_copy(
                    QT_z[h][r0:r0 + 64, qb * 512:(qb + 1) * 512],
                    psq[r0:r0 + 64, 0:512])

        def emit_kproj(qt, m):
            psm = ps.tile([128, 512], F32, tag="o", bufs=2, name=f"psk_{qt}_{m}")
            for c in range(DC):
                xsl = xk_res[:, qt, c * 512:(c + 1) * 512]
                nc.tensor.matmul(
                    psm[:, 0:512], ak_t[:, c, m * 128:(m + 1) * 128], xsl,
                    start=(c == 0), stop=(c == DC - 1 and not use_bias))
            if use_bias:
                nc.tensor.matmul(
                    psm[:, 0:512], bk_t[:, m * 128:(m + 1) * 128],
                    ones_t[0:1, :], start=False, stop=True)
            nc.vector.tensor_copy(
                KT_sb[m][:, qt * 512:(qt + 1) * 512], psm[:, 0:512])

        def emit_vproj(ktg, j):
            psv = ps.tile([128, 512], F32, tag="o", bufs=2, name=f"psv_{ktg}_{j}")
            for c in range(DC):
                xsl = xv_res[:, ktg, c * 512:(c + 1) * 512]
                nc.tensor.matmul(
                    psv[:, 0:DG], xsl[:, j * 128:(j + 1) * 128],
                    av_t[:, c, :],
                    start=(c == 0), stop=(c == DC - 1 and not use_bias))
            if use_bias:
                nc.tensor.matmul(
                    psv[:, 0:DG], ones_t[0:1, 0:128], bv_t,
                    start=False, stop=True)
            kt = ktg * 4 + j
            srcv = psv[:, 0:DG].rearrange("p (h d) -> p h d", h=GH)
            if use_mask:
                nc.vector.tensor_scalar_mul(
                    V1[:, kt, :, 0:HD], srcv, mask_t[:, kt:kt + 1])
            else:
                nc.vector.tensor_copy(V1[:, kt, :, 0:HD], srcv)

        # ---- the attention round engine ----
        # round r = (qb, hp, sk, hh): S^T (2 MMs -> pss), exp, then PV of
        # the PREVIOUS round (software pipeline, 1-round lag).
        filler = []           # list of closures, each ~2 matmuls
        fill_debt = [0.0]     # fractional chunks owed

        def pop_filler(n=1.0):
            fill_debt[0] += n
            while fill_debt[0] >= 1.0 and filler:
                filler.pop(0)()
                fill_debt[0] -= 1.0

        def emit_st(qb, hp, sk, hh):
            h = hp * 2 + hh
            pss = ps.tile([128, 1024], F32, tag="s", bufs=2,
                          name=f"pss_{qb}_{sk}_{h}")
            for dk in range(2):
                kt = sk * 2 + dk
                nc.tensor.matmul(
                    pss[:, dk * 512:(dk + 1) * 512],
                    KT_sb[hp][:, kt * 128:(kt + 1) * 128],
                    QT_z[h][:, qb * 512:qb * 512 + QB],
                    start=True, stop=True)
            pt = sb_pt.tile([128, 1024], F16, tag="pt", bufs=6,
                            name=f"pt_{qb}_{sk}_{h}")
            nc.scalar.activation(out=pt, in_=pss, func=AF.Exp, scale=SCALE)
            return pt

        def emit_pv(qb, hp, sk, hh, pt, pso):
            h = hp * 2 + hh
            for dk in range(2):
                kt = sk * 2 + dk
                nc.tensor.matmul(
                    pso[hh][0:HD + 1, :], V1[:, kt, h, :],
                    pt[:, dk * 512:(dk + 1) * 512],
                    start=(kt == 0), stop=(kt == KT - 1))

        def emit_hp_tail(qb, hp, pso):
            # normalize both heads of the pair into oN2[qb%2][hp];
            # the two heads' chains are interleaved so engines pipeline
            on = oN2[qb % 2][hp]
            oTs, bcs, rcps = [], [], []
            for hh in range(2):
                oT = sb_n.tile([HD + 1, 512], F16, tag="oT", bufs=4,
                               name=f"oT_{qb}_{hp}_{hh}")
                nc.vector.tensor_copy(oT, pso[hh][0:HD + 1, :])
                oTs.append(oT)
            for hh in range(2):
                bc = ps.tile([128, 512], F32, tag="o", bufs=2,
                             name=f"bc_{qb}_{hp}_{hh}")
                nc.tensor.matmul(
                    bc[0:64, :], ones_t[64:65, 0:64], oTs[hh][64:65, :],
                    start=True, stop=True, tile_position=(64, 0))
                bcs.append(bc)
            for hh in range(2):
                rcp = sb_n.tile([64, 512], F32, tag="rcp", bufs=2,
                                name=f"rcp_{qb}_{hp}_{hh}")
                nc.vector.reciprocal_approx_fast(out=rcp, in_=bcs[hh][0:64, :])
                rcps.append(rcp)
            for hh in range(2):
                rows = slice(0, 64) if hh == 0 else slice(64, 128)
                nc.vector.tensor_tensor(on[rows, :], oTs[hh][0:64, :],
                                        rcps[hh], op=MULT)

        def emit_outproj_chunk(qb, mq, nb, psout_box, tag="o"):
            # two packed MMs (hp 0,1) accumulating psout, then CAST out
            on_pair = oN2[qb % 2]
            psout = ps.tile([128, 512], F32, tag=tag, bufs=2,
                            name=f"psout_{qb}_{mq}_{nb}")
            for hp in range(2):
                nc.tensor.matmul(
                    psout[:, 0:512],
                    on_pair[hp][:, mq * 128:(mq + 1) * 128],
                    bo_t[:, hp, nb * 512:(nb + 1) * 512],
                    start=(hp == 0), stop=(hp == 1))
            psout_box[nb] = psout

        def emit_outproj(qb, deep=False):
            # returns filler closures: 8 chunks of 2 MMs + CAST/DMA.
            # deep=True (endgame): alternate psum tags "o"/"s" so 4 psout
            # tiles pipeline (the attention rings are drained by then).
            chunks = []
            for mq in range(4):
                ot = sb_out.tile([128, D], F16, tag="ot", name=f"ot_{qb}_{mq}")
                box = {}
                tg0 = "s" if deep and mq % 2 else "o"
                tg1 = "s" if deep and not mq % 2 else "o"

                def mk(qb=qb, mq=mq, ot=ot, box=box, tg0=tg0, tg1=tg1):
                    def c0():
                        emit_outproj_chunk(qb, mq, 0, box, tg0)
                        nc.vector.tensor_copy(ot[:, 0:512], box[0][:, 0:512])

                    def c1():
                        emit_outproj_chunk(qb, mq, 1, box, tg1)
                        nc.vector.tensor_copy(ot[:, 512:1024], box[1][:, 0:512])
                        q0 = qb * QB + mq * 128
                        nc.gpsimd.dma_start(out=t["outp"][q0:q0 + 128, :],
                                            in_=ot)
                    return [c0, c1]
                chunks.extend(mk())
            return chunks

        # ---- schedule ----
        # Minimal prelude: only what round 0 of qb0/hp0 needs (heads 0-1's
        # Q and K). Everything else is round filler, so the ScalarE exp
        # stream — the secondary pacer — starts ~20us earlier.
        emit_qproj(0, 0)
        emit_kproj(0, 0)

        # qb0 filler maps (round index -> closures), deadline-correct for
        # the 2-round PV lag: S^T(sk)@round 2sk needs kproj(sk//2,0) in an
        # earlier round; PV(sk,h0)@round 2sk+2 needs vproj up to kt=2sk+1
        # by that round's filler slot (filler precedes the PV).
        sched_q0h0 = {
            0: [lambda: emit_vproj(0, 0)],
            1: [lambda: emit_vproj(0, 1)],
            2: [lambda: emit_vproj(0, 2)],
            3: [lambda: emit_kproj(1, 0)],
            4: [lambda: emit_vproj(0, 3)],
            5: [lambda: emit_vproj(1, 0)],
            6: [lambda: emit_vproj(1, 1)],
            7: [lambda: emit_kproj(2, 0)],
            8: [lambda: emit_vproj(1, 2), lambda: emit_vproj(1, 3)],
            9: [lambda: emit_vproj(2, 0)],
            10: [lambda: emit_vproj(2, 1)],
            11: [lambda: emit_kproj(3, 0)],
            12: [lambda: emit_vproj(2, 2), lambda: emit_vproj(2, 3)],
            13: [lambda: emit_vproj(3, 0)],
            14: [lambda: emit_vproj(3, 1)],
            15: [lambda: emit_vproj(3, 2), lambda: emit_vproj(3, 3)],
        }
        sched_q0h1 = {
            0: [lambda: emit_kproj(1, 1)],
            1: [lambda: emit_qproj(1, 0)],
            3: [lambda: emit_kproj(2, 1)],
            5: [lambda: emit_qproj(1, 1)],
            7: [lambda: emit_kproj(3, 1)],
            9: [lambda: emit_qproj(2, 0)],
        }

        def spread(items, n_rounds=32, reserve=2):
            """Assign items evenly to round indices, keeping `reserve`
            items back for the hp-tail boundaries."""
            body = items[:len(items) - reserve] if reserve else items
            tail = items[len(items) - reserve:] if reserve else []
            m = {}
            if body:
                for i, it in enumerate(body):
                    m.setdefault(i * n_rounds // len(body), []).append(it)
            return m, tail

        def run_qb(qb, s0, s1, t0, t1):
            for hp in range(2):
                sched = s0 if hp == 0 else s1
                tailf = t0 if hp == 0 else t1
                pso = [ps.tile([128, 512], F32, tag="acc", bufs=2,
                               name=f"pso_{qb}_{hp}_{hh}") for hh in range(2)]
                pend = []
                ridx = 0
                for sk in range(8):
                    for hh in range(2):
                        # order: S^T(r), filler, PV(r-2) — a 2-round PV lag
                        # so the PV's exp finished >1 round ago and the PE
                        # never waits on a fresh semaphore
                        pt = emit_st(qb, hp, sk, hh)
                        for fn in sched.pop(ridx, []):
                            fn()
                        if len(pend) == 2:
                            emit_pv(*pend.pop(0))
                        pend.append((qb, hp, sk, hh, pt, pso))
                        ridx += 1
                for p in pend:
                    emit_pv(*p)
                for fn in tailf:
                    fn()
                emit_hp_tail(qb, hp, pso)

        run_qb(0, sched_q0h0, sched_q0h1,
               [lambda: emit_qproj(0, 1), lambda: emit_kproj(0, 1)],
               [lambda: emit_qproj(2, 1)])
        for qb in range(1, QT):
            items = emit_outproj(qb - 1)
            if qb == 2:
                items += [lambda: emit_qproj(3, 0), lambda: emit_qproj(3, 1)]
            half0, rest = items[:len(items) // 2], items[len(items) // 2:]
            s0, t0 = spread(half0, n_rounds=16, reserve=1)
            s1, t1 = spread(rest, n_rounds=16, reserve=2 if qb == QT - 1 else 1)
            run_qb(qb, s0, s1, t0, t1)
        for ch in emit_outproj(QT - 1, deep=True):
            ch()


def _swizzle_a(aT):
    """[D, DG] -> [128, DC*DG]: partition p holds chunks c at (c, :)."""
    return np.ascontiguousarray(
        aT.reshape(DC, 128, DG).transpose(1, 0, 2).reshape(128, DC * DG))


def _pack_bo(boT):
    """[DG, D] -> [128, 2*D]: head-pair hp at cols hp*D, rows=pair dims."""
    out = boT.reshape(2, 128, D).transpose(1, 0, 2)
    return np.ascontiguousarray(out.reshape(128, 2 * D))


def _block_x(xT):
    """[D, L] -> [QT*128, DC*512] qt-major blocks, 8KB partition lines."""
    return np.ascontiguousarray(
        xT.reshape(DC, 128, QT, 512).transpose(2, 1, 0, 3).reshape(
            QT * 128, DC * 512))


_EYE = np.eye(64, dtype=np.float16)


def _prep_inputs(values, key, query, mask, Wv, Wk, Wq, Wo, bv, bk, bq):
    """Build the 8 per-core input maps (host-side shard + layout)."""
    xB = {}
    for n in range(N_BATCH):
        xB[("q", n)] = _block_x(query[n].T.astype(np.float16))
        xB[("k", n)] = _block_x(key[n].T.astype(np.float16))
        xB[("v", n)] = _block_x(values[n].T.astype(np.float16))
    in_maps = []
    for c in range(CORES):
        n, g = divmod(c, CORES // N_BATCH)
        rows = slice(g * DG, (g + 1) * DG)
        mrow = np.ascontiguousarray(
            mask[n, 0, 0, :].astype(np.float32).reshape(KT, 128).T)
        in_maps.append({
            "xqb": xB[("q", n)],
            "xkb": xB[("k", n)],
            "xvb": xB[("v", n)],
            "aq": _swizzle_a(Wq[rows, :].T.astype(np.float16)),
            "ak": _swizzle_a(Wk[rows, :].T.astype(np.float16)),
            "av": _swizzle_a(Wv[rows, :].T.astype(np.float16)),
            "bo": _pack_bo(Wo[:, rows].T.astype(np.float16)),
            "bq": np.ascontiguousarray(bq[None, rows].astype(np.float16)),
            "bk": np.ascontiguousarray(bk[None, rows].astype(np.float16)),
            "bv": np.ascontiguousarray(bv[None, rows].astype(np.float16)),
            "eye": _EYE,
            "maskf": mrow,
        })
    return in_maps


LAST_EXEC_NS = None
LAST_RES = None


def kernel(values, key, query, mask, Wv, bv, Wk, bk, Wq, bq, Wo, bo,
           trace=False, trace_cores=None):
    global LAST_EXEC_NS, LAST_RES
    values = np.asarray(values, dtype=np.float32)
    key = np.asarray(key, dtype=np.float32)
    query = np.asarray(query, dtype=np.float32)
    mask = np.asarray(mask)
    Wq, Wk, Wv, Wo = (np.asarray(Wq, np.float32), np.asarray(Wk, np.float32),
                      np.asarray(Wv, np.float32), np.asarray(Wo, np.float32))
    bq, bk, bv, bo = (np.asarray(bq, np.float32), np.asarray(bk, np.float32),
                      np.asarray(bv, np.float32), np.asarray(bo, np.float32))

    use_bias = bool(np.any(bq) or np.any(bk) or np.any(bv))
    use_mask = not bool(np.all(np.asarray(mask) == 1))

    nc = _build(use_bias, use_mask)
    in_maps = _prep_inputs(values, key, query, mask, Wv, Wk, Wq, Wo,
                           bv, bk, bq)
    kw = {}
    if trace_cores is not None:
        kw["trace_cores"] = trace_cores
    res = run_bass_kernel_spmd(nc, in_maps, core_ids=list(range(CORES)),
                               trace=trace, **kw)
    LAST_EXEC_NS = res.exec_time_ns
    LAST_RES = res

    out = np.zeros((N_BATCH, L, D), dtype=np.float32)
    for c in range(CORES):
        n = c // (CORES // N_BATCH)
        out[n] += res.results[c]["outp"].astype(np.float32)
    out += bo[None, None, :]
    return out
